# revision 1
# baseline (speedup 1.0000x reference)
"""Trainium2 Bass kernel for the forward-attention LSA step (nn_LSA_43404939494068).

Contract: kernel(**inputs) takes the FULL inputs from setup_inputs() and
returns the FULL output [64, 1, 1024] float32. Internally shards batch
across 8 NeuronCores (8 batches each), runs one Bass/Tile program SPMD.

Math notes (vs reference):
  u[b,t]   = sum_a v[a] * tanh(pq[b,a] + enc[b,t,a] + ploc[b,t,a])
  ploc     = conv1d([cumulative; attention]) @ L_w.T + L_b; the conv and the
             L-projection fold into ONE matmul: ploc[t,:] = ls[:,t].T @ M,
             M[j,a] = sum_f conv_w[f,c_j,k_j] * L_w[a,f] (host-precomputed
             weight algebra), ls = 62 shifted copies of the two loc rows
             (+ a ones row that carries pq+W_b+L_b into the same matmul).
  The reference's division of s=sigmoid(u) by sum(s) cancels exactly in the
  final alpha normalization, so it is skipped.

Per [128t x 512a] tile: PE matmul (K=63: folded conv+proj+bias) and PE
identity-matmul (accumulates enc into the same PSUM bank), ACT tanh, DVE
tensor_tensor_reduce (x*v and row-sum fused) -> one u column. The tail
(sigmoid, mask, alpha-shift recursion, normalize) runs in a [t',(k,b)]
64-column layout with the t-shifts expressed as two small band-matrix
matmuls, then one PE transpose and a single output DMA.
"""

import sys

import numpy as np

if "/opt/trn_rl_repo" not in sys.path:
    sys.path.insert(0, "/opt/trn_rl_repo")

import concourse.bass as bass
import concourse.tile as tile
from concourse import mybir
from concourse.bass_utils import run_bass_kernel_spmd

B, T, A = 64, 1024, 512
F, KW = 32, 31
PAD = (KW - 1) // 2
NCORES = 8
LB = B // NCORES          # 8 local batches per core
NK = T // 128             # 8 t-tiles of 128
KC = 62                   # conv contraction = 2 channels * 31 taps
KP = 640                  # 513 padded to 5*128 for the pq matmul
F32 = mybir.dt.float32

_MAX_WAITS = 1


def _split_sync_waits(nc):
    """walrus in this toolchain accepts at most one sync-wait per
    instruction; hoist excess waits onto NoOps inserted just before."""
    for fn in nc.m.functions:
        for blk in fn.blocks:
            new_list = []
            for inst in blk.instructions:
                si = inst.sync_info
                if si is not None and si.on_wait and len(si.on_wait) > _MAX_WAITS:
                    waits = list(si.on_wait)
                    extra, keep = waits[:-_MAX_WAITS], waits[-_MAX_WAITS:]
                    for i in range(0, len(extra), _MAX_WAITS):
                        nop = mybir.InstNoOp(
                            name=nc.get_next_instruction_name(),
                            sync_info=mybir.SyncInfo(
                                on_wait=extra[i:i + _MAX_WAITS], on_update=[]
                            ),
                            bass_nofuse=True,
                            engine=inst.engine,
                        )
                        nc.register_instruction(nop)
                        new_list.append(nop)
                    inst.sync_info = mybir.SyncInfo(
                        on_wait=keep, on_update=list(si.on_update)
                    )
                new_list.append(inst)
            blk.instructions[:] = new_list


def build_program(repeats: int = 1) -> bass.Bass:
    nc = bass.Bass()
    dt = F32

    enc_d = nc.declare_dram_parameter("enc", [LB, T, A], dt, isOutput=False)
    ls_d = nc.declare_dram_parameter("ls", [LB, KC + 1, T], dt, isOutput=False)
    qt_d = nc.declare_dram_parameter("qt", [KP, LB], dt, isOutput=False)
    qw_d = nc.declare_dram_parameter("qw", [KP, A], dt, isOutput=False)
    mcomb_d = nc.declare_dram_parameter("mcomb", [KC, A], dt, isOutput=False)
    vw_d = nc.declare_dram_parameter("vw", [A], dt, isOutput=False)
    eye_d = nc.declare_dram_parameter("eye", [128, 128], dt, isOutput=False)
    ones_d = nc.declare_dram_parameter("ones", [128, 128], dt, isOutput=False)
    tri_d = nc.declare_dram_parameter("tri", [128, 128], dt, isOutput=False)
    cor_d = nc.declare_dram_parameter("cor", [128, 128], dt, isOutput=False)
    mask_d = nc.declare_dram_parameter("masklay", [128, LB * NK], dt, isOutput=False)
    alpha_d = nc.declare_dram_parameter("alphalay", [128, LB * NK], dt, isOutput=False)
    out_d = nc.declare_dram_parameter("out", [LB * NK, 128], dt, isOutput=True)

    TANH = mybir.ActivationFunctionType.Tanh
    SIG = mybir.ActivationFunctionType.Sigmoid
    IDENT = mybir.ActivationFunctionType.Identity
    MULT = mybir.AluOpType.mult
    ADD = mybir.AluOpType.add

    with tile.TileContext(nc) as tc:
        with (
            tc.tile_pool(name="const", bufs=1) as cpool,
            tc.tile_pool(name="encp", bufs=2) as encp,
            tc.tile_pool(name="lsp", bufs=2) as lsp,
            tc.tile_pool(name="xp", bufs=3) as xp,
            tc.tile_pool(name="xvp", bufs=2) as xvp,
            tc.tile_pool(name="tailp", bufs=2) as tailp,
            tc.tile_pool(name="zps", bufs=4, space="PSUM") as zps,
            tc.tile_pool(name="sps", bufs=2, space="PSUM") as sps,
        ):
            # ---- constants into SBUF ----
            eye_sb = cpool.tile([128, 128], dt, tag="eye")
            nc.sync.dma_start(out=eye_sb[:], in_=eye_d[:])
            ones_sb = cpool.tile([128, 128], dt, tag="ones")
            nc.sync.dma_start(out=ones_sb[:], in_=ones_d[:])
            tri_sb = cpool.tile([128, 128], dt, tag="tri")
            nc.sync.dma_start(out=tri_sb[:], in_=tri_d[:])
            cor_sb = cpool.tile([128, 128], dt, tag="cor")
            nc.sync.dma_start(out=cor_sb[:], in_=cor_d[:])
            mask_sb = cpool.tile([128, LB * NK], dt, tag="mask")
            nc.sync.dma_start(out=mask_sb[:], in_=mask_d[:])
            alpha_sb = cpool.tile([128, LB * NK], dt, tag="alpha")
            nc.sync.dma_start(out=alpha_sb[:], in_=alpha_d[:])

            # v broadcast to all 128 partitions (partition-step-0 DMA)
            v_sb = cpool.tile([128, A], dt, tag="vbc")
            va = vw_d[:]
            v_bcast = bass.AP(tensor=va.tensor, offset=va.offset,
                              ap=[[0, 128]] + [list(p) for p in va.ap])
            nc.sync.dma_start(out=v_sb[:], in_=v_bcast)

            # pq matmul operands
            qt_sb = cpool.tile([128, KP // 128, LB], dt, tag="qt")
            nc.sync.dma_start(out=qt_sb[:],
                              in_=qt_d.rearrange("(c p) n -> p c n", p=128))
            qw_sb = cpool.tile([128, KP // 128, A], dt, tag="qw")
            nc.sync.dma_start(out=qw_sb[:],
                              in_=qw_d.rearrange("(c p) a -> p c a", p=128))

            # per-batch rhs [63, A]: rows 0..61 folded conv+proj weight,
            # row 62 = pq[b] + W_b + L_b (filled after the pq matmul)
            rhs_sb = [cpool.tile([KC + 1, A], dt, name=f"rhs{b}", tag=f"rhs{b}")
                      for b in range(LB)]
            for b in range(LB):
                nc.sync.dma_start(out=rhs_sb[b][0:KC, :], in_=mcomb_d[:])

            u_sb = cpool.tile([128, LB * NK], dt, tag="u")
            eps_sb = cpool.tile([128, 1], dt, tag="eps")
            nc.vector.memset(eps_sb[:], 1e-7)

            for rep in range(repeats):
                # ---- processed query: pq = q @ W^T + (W_b + L_b) ----
                pq_ps = sps.tile([LB, A], dt, tag="tailps")
                for i in range(KP // 128):
                    nc.tensor.matmul(pq_ps[:], qt_sb[:, i, :], qw_sb[:, i, :],
                                     start=(i == 0), stop=(i == KP // 128 - 1))
                pq_sb = tailp.tile([LB, A], dt, tag="pqsb")
                nc.scalar.copy(out=pq_sb[:], in_=pq_ps[:])
                for b in range(LB):
                    nc.sync.dma_start(out=rhs_sb[b][KC:KC + 1, :],
                                      in_=pq_sb[b:b + 1, :])

                # ---- main loop: z = ploc+pq+enc ; x = tanh(z) ; u = x.v ----
                for b in range(LB):
                    enc_sb = encp.tile([128, NK, A], dt, tag="enc")
                    nc.sync.dma_start(
                        out=enc_sb[:],
                        in_=enc_d[b].rearrange("(k p) a -> p k a", p=128))
                    ls_sb = lsp.tile([KC + 1, T], dt, tag="ls")
                    nc.sync.dma_start(out=ls_sb[:], in_=ls_d[b])
                    for k in range(NK):
                        z_ps = zps.tile([128, A], dt, tag="z")
                        nc.tensor.matmul(z_ps[:], ls_sb[:, k * 128:(k + 1) * 128],
                                         rhs_sb[b][:], start=True, stop=False)
                        nc.tensor.matmul(z_ps[:], eye_sb[:], enc_sb[:, k, :],
                                         start=False, stop=True)
                        x_sb = xp.tile([128, A], dt, tag="x")
                        nc.scalar.activation(out=x_sb[:], in_=z_ps[:], func=TANH)
                        xv_sb = xvp.tile([128, A], dt, tag="xv")
                        col = k * LB + b
                        nc.vector.scalar_tensor_tensor(
                            out=xv_sb[:], in0=x_sb[:], scalar=1.0, in1=v_sb[:],
                            op0=MULT, op1=MULT,
                            accum_out=u_sb[:, col:col + 1])

                # ---- tail in [t', (k,b)] layout ----
                s_sb = tailp.tile([128, LB * NK], dt, tag="s")
                nc.scalar.activation(out=s_sb[:], in_=u_sb[:], func=SIG)
                att_sb = tailp.tile([128, LB * NK], dt, tag="att")
                nc.vector.tensor_mul(att_sb[:], s_sb[:], mask_sb[:])

                # w = alpha + shift1(alpha) + shift2(alpha) via band matmuls
                w_ps = sps.tile([128, LB * NK], dt, tag="tailps")
                nc.tensor.matmul(w_ps[:], tri_sb[:], alpha_sb[:],
                                 start=True, stop=False)
                nc.tensor.matmul(w_ps[:, LB:], cor_sb[:], alpha_sb[:, :-LB],
                                 start=False, stop=True, skip_group_check=True)
                w_sb = tailp.tile([128, LB * NK], dt, tag="w")
                nc.scalar.activation(out=w_sb[:], in_=w_ps[:], func=IDENT,
                                     bias=eps_sb[:], scale=1.0)

                na_sb = tailp.tile([128, LB * NK], dt, tag="na")
                nc.vector.tensor_mul(na_sb[:], att_sb[:], w_sb[:])

                # per-batch normalizer: colsum then sum over the k-groups
                cs_ps = sps.tile([1, LB * NK], dt, tag="tailps")
                nc.tensor.matmul(cs_ps[:], ones_sb[:, 0:1], na_sb[:],
                                 start=True, stop=True)
                z_sb = tailp.tile([1, LB], dt, tag="zsum")
                nc.vector.tensor_reduce(
                    out=z_sb[:], in_=cs_ps.rearrange("p (k b) -> p b k", b=LB),
                    axis=mybir.AxisListType.X, op=ADD)
                rz_sb = tailp.tile([1, LB], dt, tag="rz")
                nc.vector.reciprocal(out=rz_sb[:], in_=z_sb[:])
                rz64_sb = tailp.tile([1, LB * NK], dt, tag="rz64")
                rza = rz_sb[:]
                rz_b = bass.AP(tensor=rza.tensor, offset=rza.offset,
                               ap=[list(rza.ap[0]), [0, NK], list(rza.ap[1])])
                nc.vector.tensor_copy(
                    out=rz64_sb.rearrange("p (k b) -> p k b", b=LB), in_=rz_b)
                rb_ps = sps.tile([128, LB * NK], dt, tag="tailps")
                nc.tensor.matmul(rb_ps[:], ones_sb[0:1, :], rz64_sb[:],
                                 start=True, stop=True)
                nan_sb = tailp.tile([128, LB * NK], dt, tag="nan")
                nc.vector.tensor_mul(nan_sb[:], na_sb[:], rb_ps[:])

                # transpose to [(k b), t'] and store
                ot_ps = sps.tile([LB * NK, 128], dt, tag="tailps")
                nc.tensor.transpose(ot_ps[:], nan_sb[:], eye_sb[:])
                ot_sb = tailp.tile([LB * NK, 128], dt, tag="otsb")
                nc.scalar.copy(out=ot_sb[:], in_=ot_ps[:])
                nc.sync.dma_start(out=out_d[:], in_=ot_sb[:])

    _split_sync_waits(nc)
    return nc


def prep_inputs(inputs: dict) -> list[dict]:
    """Full inputs -> per-core in_maps (host layout prep only)."""
    enc = np.asarray(inputs["encoder_seq_proj"], np.float32)
    query = np.asarray(inputs["query"], np.float32)
    cum = np.asarray(inputs["cumulative"], np.float32)
    att = np.asarray(inputs["attention"], np.float32)
    alpha = np.asarray(inputs["alpha"], np.float32)
    conv_w = np.asarray(inputs["conv_w"], np.float32)
    L_w = np.asarray(inputs["L_w"], np.float32)
    L_b = np.asarray(inputs["L_b"], np.float32)
    W_w = np.asarray(inputs["W_w"], np.float32)
    W_b = np.asarray(inputs["W_b"], np.float32)
    v_w = np.asarray(inputs["v_w"], np.float32)
    phone_len = np.asarray(inputs["phone_len"], np.int64)

    # folded conv+projection weight: M[c*31+k, a] = sum_f conv_w[f,c,k]*L_w[a,f]
    mcomb = np.einsum("fck,af->cka", conv_w, L_w).reshape(KC, A)
    mcomb = np.ascontiguousarray(mcomb, np.float32)

    qw = np.zeros((KP, A), np.float32)
    qw[:A] = W_w.T
    qw[A] = W_b + L_b

    eye = np.eye(128, dtype=np.float32)
    ones = np.ones((128, 128), np.float32)
    # tri[s,t'] = 1 for t'-2 <= s <= t'  (alpha + shift1 + shift2, in-block)
    idx = np.arange(128)
    dif = idx[None, :] - idx[:, None]          # t' - s
    tri = ((dif >= 0) & (dif <= 2)).astype(np.float32)
    # cor[s,t']: cross-block corner terms from the previous 128-block
    cor = np.zeros((128, 128), np.float32)
    cor[126, 0] = 1.0
    cor[127, 0] = 1.0
    cor[127, 1] = 1.0

    mask = (np.arange(T)[None, :] < phone_len[:, None]).astype(np.float32)

    def lay(arr):  # [8,1024] -> [128, 64] with col = k*8 + b
        return np.ascontiguousarray(
            arr.reshape(LB, NK, 128).transpose(2, 1, 0).reshape(128, LB * NK))

    in_maps = []
    for c in range(NCORES):
        sl = slice(c * LB, (c + 1) * LB)
        cum_c, att_c = cum[sl], att[sl]
        ls = np.zeros((LB, KC + 1, T), np.float32)
        padc = np.zeros((LB, T + 2 * PAD), np.float32)
        pada = np.zeros((LB, T + 2 * PAD), np.float32)
        padc[:, PAD:PAD + T] = cum_c
        pada[:, PAD:PAD + T] = att_c
        for k in range(KW):
            ls[:, k, :] = padc[:, k:k + T]
            ls[:, KW + k, :] = pada[:, k:k + T]
        ls[:, KC, :] = 1.0

        qt = np.zeros((KP, LB), np.float32)
        qt[:A] = query[sl].T
        qt[A] = 1.0

        in_maps.append({
            "enc": np.ascontiguousarray(enc[sl]),
            "ls": ls,
            "qt": qt,
            "qw": qw,
            "mcomb": mcomb,
            "vw": np.ascontiguousarray(v_w[0]),
            "eye": eye,
            "ones": ones,
            "tri": tri,
            "cor": cor,
            "masklay": lay(mask[sl]),
            "alphalay": lay(alpha[sl]),
        })
    return in_maps


def assemble_output(results: list[dict]) -> np.ndarray:
    out = np.empty((B, 1, T), np.float32)
    for c in range(NCORES):
        oc = results[c]["out"]                      # [(k b), 128]
        oc = oc.reshape(NK, LB, 128).transpose(1, 0, 2).reshape(LB, T)
        out[c * LB:(c + 1) * LB, 0, :] = oc
    return out


_CACHED_NC = None


def kernel(**inputs) -> np.ndarray:
    global _CACHED_NC
    if _CACHED_NC is None:
        _CACHED_NC = build_program(repeats=1)
    in_maps = prep_inputs(inputs)
    res = run_bass_kernel_spmd(_CACHED_NC, in_maps, list(range(NCORES)))
    return assemble_output(res.results)



# revision 11
# speedup vs baseline: 1006.8302x; 1006.8302x over previous
"""Trainium2 Bass kernel for the forward-attention LSA step (nn_LSA_43404939494068).

Contract: kernel(**inputs) takes the FULL inputs from setup_inputs() and
returns the FULL output [64, 1, 1024] float32. Internally shards batch
across 8 NeuronCores (8 batches each), runs one Bass/Tile program SPMD.

Math notes (vs reference):
  u[b,t]   = sum_a v[a] * tanh(pq[b,a] + enc[b,t,a] + ploc[b,t,a])
  ploc     = conv1d([cumulative; attention]) @ L_w.T + L_b; the conv and the
             L-projection fold into ONE matmul: ploc[t,:] = ls[:,t].T @ M,
             M[(c,k),a] = sum_f conv_w[f,c,k] * L_w[a,f] (host-precomputed
             weight algebra), ls = 62 shifted copies of the two loc rows.
  pq+L_b+W_b is computed on host (tiny: B x A) and folded into enc, which is
  shipped bf16 (halves HBM traffic; final rel-err stays ~1e-3 << 2e-2).
  The reference's division of s=sigmoid(u) by sum(s) cancels exactly in the
  final alpha normalization, so it is skipped.

Engine placement per [128t x 512a] tile: PE matmul in float32r (1 cyc/row
at N=512 vs 4 for plain fp32) of the folded conv+proj, plus a bf16
identity-matmul accumulating enc into the same PSUM bank; ACT tanh
(PSUM->SBUF bf16); the x*v dot runs on the otherwise-idle Pool/GpSimd
engine (scalar_tensor_tensor with accum) writing one u column. The tail
(sigmoid, mask, alpha-shift recursion via band matmuls, normalize) runs in
a [t',(k,b)] 64-column layout on DVE/PE/ACT, then one PE transpose and a
single output DMA. Input DMAs are spread across engine queues (enc on SP,
the rest on DVE) so the big enc stream owns its ring.
"""

import sys

import numpy as np

if "/opt/trn_rl_repo" not in sys.path:
    sys.path.insert(0, "/opt/trn_rl_repo")

import concourse.bass as bass
import concourse.tile as tile
from concourse import mybir
from concourse.bass_utils import run_bass_kernel_spmd

B, T, A = 64, 1024, 512
F, KW = 32, 31
PAD = (KW - 1) // 2
NCORES = 8
LB = B // NCORES          # 8 local batches per core
NK = T // 128             # 8 t-tiles of 128
KC = 62                   # conv contraction = 2 channels * 31 taps
F32 = mybir.dt.float32
F32R = mybir.dt.float32r
BF16 = mybir.dt.bfloat16

# const blob layout (fp32, [128, 640]): tri | cor | ones | eye32 | mask | alpha
C_TRI, C_COR, C_ONES, C_EYE = 0, 128, 256, 384
C_MASK, C_ALPHA = 512, 576
C_W = 640

_MAX_WAITS = 1


def _split_sync_waits(nc):
    """walrus in this toolchain accepts at most one sync-wait per
    instruction; hoist excess waits onto NoOps inserted just before."""
    for fn in nc.m.functions:
        for blk in fn.blocks:
            new_list = []
            for inst in blk.instructions:
                si = inst.sync_info
                if si is not None and si.on_wait and len(si.on_wait) > _MAX_WAITS:
                    waits = list(si.on_wait)
                    extra, keep = waits[:-_MAX_WAITS], waits[-_MAX_WAITS:]
                    for i in range(0, len(extra), _MAX_WAITS):
                        nop = mybir.InstNoOp(
                            name=nc.get_next_instruction_name(),
                            sync_info=mybir.SyncInfo(
                                on_wait=extra[i:i + _MAX_WAITS], on_update=[]
                            ),
                            bass_nofuse=True,
                            engine=inst.engine,
                        )
                        nc.register_instruction(nop)
                        new_list.append(nop)
                    inst.sync_info = mybir.SyncInfo(
                        on_wait=keep, on_update=list(si.on_update)
                    )
                new_list.append(inst)
            blk.instructions[:] = new_list


def build_program(repeats: int = 1, hw_loop: bool = False,
                  stage: str = "full") -> bass.Bass:
    nc = bass.Bass()

    enc_d = nc.declare_dram_parameter("enc", [LB, T, A], BF16, isOutput=False)
    ls_d = nc.declare_dram_parameter("ls", [KC, LB, T], BF16, isOutput=False)
    mcomb_d = nc.declare_dram_parameter("mcomb", [KC, A], BF16, isOutput=False)
    vw_d = nc.declare_dram_parameter("vw", [A], BF16, isOutput=False)
    eyeb_d = nc.declare_dram_parameter("eyeb", [128, 128], BF16, isOutput=False)
    const_d = nc.declare_dram_parameter("constblob", [128, C_W], F32, isOutput=False)
    out_d = nc.declare_dram_parameter("out", [LB * NK, 128], F32, isOutput=True)

    TANH = mybir.ActivationFunctionType.Tanh
    SIG = mybir.ActivationFunctionType.Sigmoid
    IDENT = mybir.ActivationFunctionType.Identity
    MULT = mybir.AluOpType.mult
    ADD = mybir.AluOpType.add

    with tile.TileContext(nc) as tc:
        with (
            tc.tile_pool(name="const", bufs=1) as cpool,
            tc.tile_pool(name="encp", bufs=3) as encp,
            tc.tile_pool(name="xp", bufs=3) as xp,
            tc.tile_pool(name="xvp", bufs=2) as xvp,
            tc.tile_pool(name="tailp", bufs=2) as tailp,
            tc.tile_pool(name="zps", bufs=3, space="PSUM") as zps,
            tc.tile_pool(name="sps", bufs=2, space="PSUM") as sps,
        ):
            # ---- constants into SBUF (gpsimd/SWDGE queue; enc owns the SP
            # queue). Order = first-use order: the b0 matmuls need mcomb+eyeb
            # +ls[0] immediately; the const blob is tail-only.
            mcomb_sb = cpool.tile([KC, A], BF16, tag="mcomb")
            nc.scalar.dma_start(out=mcomb_sb[:], in_=mcomb_d[:])
            eyeb_sb = cpool.tile([128, 128], BF16, tag="eyeb")
            nc.scalar.dma_start(out=eyeb_sb[:], in_=eyeb_d[:])

            ls_sb = cpool.tile([KC, LB, T], BF16, tag="ls")
            nc.scalar.dma_start(out=ls_sb[:, 0, :], in_=ls_d[:, 0, :])

            # v broadcast to all 128 partitions (partition-step-0 DMA)
            v_sb = cpool.tile([128, A], BF16, tag="vbc")
            va = vw_d[:]
            v_bcast = bass.AP(tensor=va.tensor, offset=va.offset,
                              ap=[[0, 128]] + [list(p) for p in va.ap])
            nc.scalar.dma_start(out=v_sb[:], in_=v_bcast)

            for b in range(1, LB):
                nc.gpsimd.dma_start(out=ls_sb[:, b, :], in_=ls_d[:, b, :])
            const_sb = cpool.tile([128, C_W], F32, tag="const")
            nc.gpsimd.dma_start(out=const_sb[:], in_=const_d[:])

            u_sb = cpool.tile([128, LB * NK], F32, tag="u")
            eps_sb = cpool.tile([128, 1], F32, tag="eps")
            nc.vector.memset(eps_sb[:], 1e-7)
            warm_sb = cpool.tile([128, 1], F32, tag="warm")
            nc.scalar.activation(out=warm_sb[:], in_=eps_sb[:], func=TANH)

            tri = const_sb[:, C_TRI:C_TRI + 128]
            cor = const_sb[:, C_COR:C_COR + 128]
            ones = const_sb[:, C_ONES:C_ONES + 128]
            eye32 = const_sb[:, C_EYE:C_EYE + 128]
            mask = const_sb[:, C_MASK:C_MASK + LB * NK]
            alpha = const_sb[:, C_ALPHA:C_ALPHA + LB * NK]

            def body():
                # ---- main loop: z = ploc+(pq+enc) ; x = tanh(z) ; u = x.v ----
                for b in range(LB):
                    enc_sb = encp.tile([128, NK, A], BF16, tag="enc")
                    src_enc = enc_d[b].rearrange("(k p) a -> p k a", p=128)
                    if b == 0:
                        nc.sync.dma_start(out=enc_sb[:, 0:2, :],
                                          in_=src_enc[:, 0:2, :])
                        nc.sync.dma_start(out=enc_sb[:, 2:, :],
                                          in_=src_enc[:, 2:, :])
                    else:
                        nc.sync.dma_start(out=enc_sb[:], in_=src_enc)
                    if stage == "dma":
                        continue
                    for kp in range(NK // 2):
                        # two k-tiles share one 2-bank PSUM tile so a single
                        # tanh covers both (amortizes ACT access latency)
                        # dve_add pairs: enc joins via a DVE add instead of
                        # the PE identity-matmul (balances PE vs DVE load)
                        dve_add = (stage in ("full", "fullh") and kp % 2 == 1)
                        z_ps = zps.tile([128, 2, A], F32, tag="z")
                        for j in range(2):
                            k = 2 * kp + j
                            nc.tensor.matmul(
                                z_ps[:, j, :],
                                ls_sb[:, b, k * 128:(k + 1) * 128],
                                mcomb_sb[:],
                                start=True, stop=(dve_add or stage == "noeye"))
                            if not (dve_add or stage == "noeye"):
                                nc.tensor.matmul(z_ps[:, j, :], eyeb_sb[:],
                                                 enc_sb[:, k, :],
                                                 start=False, stop=True)
                        if stage == "mm":
                            continue
                        x_sb = xp.tile([128, 2, A], BF16, tag="x")
                        if dve_add:
                            xin_sb = xp.tile([128, 2, A], F32, tag="xin")
                            nc.vector.tensor_add(
                                xin_sb[:], z_ps[:],
                                enc_sb[:, 2 * kp:2 * kp + 2, :])
                            nc.scalar.activation(out=x_sb[:], in_=xin_sb[:],
                                                 func=TANH)
                        else:
                            nc.scalar.activation(out=x_sb[:], in_=z_ps[:],
                                                 func=TANH)
                        if stage == "act":
                            continue
                        for j in range(2):
                            k = 2 * kp + j
                            xv_sb = xvp.tile([128, A], BF16, tag="xv")
                            col = k * LB + b
                            nc.vector.scalar_tensor_tensor(
                                out=xv_sb[:], in0=x_sb[:, j, :], scalar=1.0,
                                in1=v_sb[:], op0=MULT, op1=MULT,
                                accum_out=u_sb[:, col:col + 1])

                if stage != "full":
                    dum_sb = tailp.tile([LB * NK, 128], F32, tag="otsb")
                    nc.vector.tensor_copy(out=dum_sb[:],
                                          in_=const_sb[0:LB * NK, 0:128])
                    nc.sync.dma_start(out=out_d[:], in_=dum_sb[:])
                    return
                # ---- tail in [t', (k,b)] layout ----
                # wm = (alpha + shift1(alpha) + shift2(alpha) + eps) * mask
                # depends only on constants: runs under the main loop, off
                # the post-u critical path (band shifts via tri/cor matmuls)
                w_ps = sps.tile([128, LB * NK], F32, tag="tailps")
                nc.tensor.matmul(w_ps[:], tri, alpha, start=True, stop=False)
                nc.tensor.matmul(w_ps[:, LB:], cor, alpha[:, :-LB],
                                 start=False, stop=True, skip_group_check=True)
                wm_sb = tailp.tile([128, LB * NK], F32, tag="wm")
                nc.vector.scalar_tensor_tensor(
                    out=wm_sb[:], in0=w_ps[:], scalar=1e-7, in1=mask,
                    op0=ADD, op1=MULT)

                s_sb = tailp.tile([128, LB * NK], F32, tag="s")
                nc.scalar.activation(out=s_sb[:], in_=u_sb[:], func=SIG)
                na_sb = tailp.tile([128, LB * NK], F32, tag="na")
                nc.vector.tensor_mul(na_sb[:], s_sb[:], wm_sb[:])

                # per-batch normalizer: colsum then sum over the k-groups
                cs_ps = sps.tile([1, LB * NK], F32, tag="tailps")
                nc.tensor.matmul(cs_ps[:], ones[:, 0:1], na_sb[:],
                                 start=True, stop=True)
                z_sb = tailp.tile([1, LB], F32, tag="zsum")
                nc.vector.tensor_reduce(
                    out=z_sb[:], in_=cs_ps.rearrange("p (k b) -> p b k", b=LB),
                    axis=mybir.AxisListType.X, op=ADD)
                rz_sb = tailp.tile([1, LB], F32, tag="rz")
                nc.vector.reciprocal(out=rz_sb[:], in_=z_sb[:])
                rz64_sb = tailp.tile([1, LB * NK], F32, tag="rz64")
                rza = rz_sb[:]
                rz_b = bass.AP(tensor=rza.tensor, offset=rza.offset,
                               ap=[list(rza.ap[0]), [0, NK], list(rza.ap[1])])
                nc.vector.tensor_copy(
                    out=rz64_sb.rearrange("p (k b) -> p k b", b=LB), in_=rz_b)
                rb_ps = sps.tile([128, LB * NK], F32, tag="tailps")
                nc.tensor.matmul(rb_ps[:], ones[0:1, :], rz64_sb[:],
                                 start=True, stop=True)
                nan_sb = tailp.tile([128, LB * NK], F32, tag="nan")
                nc.vector.tensor_mul(nan_sb[:], na_sb[:], rb_ps[:])

                # transpose to [(k b), t'] and store
                ot_ps = sps.tile([LB * NK, 128], F32, tag="tailps")
                nc.tensor.transpose(ot_ps[:], nan_sb[:], eye32)
                ot_sb = tailp.tile([LB * NK, 128], F32, tag="otsb")
                nc.vector.tensor_copy(out=ot_sb[:], in_=ot_ps[:])
                nc.sync.dma_start(out=out_d[:], in_=ot_sb[:])

            if hw_loop and repeats > 1:
                with tc.For_i(0, repeats, 1):
                    body()
            else:
                for _rep in range(repeats):
                    body()

    _split_sync_waits(nc)
    return nc


def prep_inputs(inputs: dict) -> list[dict]:
    """Full inputs -> per-core in_maps (host layout prep only)."""
    import ml_dtypes

    enc = np.asarray(inputs["encoder_seq_proj"], np.float32)
    query = np.asarray(inputs["query"], np.float32)
    cum = np.asarray(inputs["cumulative"], np.float32)
    att = np.asarray(inputs["attention"], np.float32)
    alpha = np.asarray(inputs["alpha"], np.float32)
    conv_w = np.asarray(inputs["conv_w"], np.float32)
    L_w = np.asarray(inputs["L_w"], np.float32)
    L_b = np.asarray(inputs["L_b"], np.float32)
    W_w = np.asarray(inputs["W_w"], np.float32)
    W_b = np.asarray(inputs["W_b"], np.float32)
    v_w = np.asarray(inputs["v_w"], np.float32)
    phone_len = np.asarray(inputs["phone_len"], np.int64)

    # folded conv+projection weight: M[c*31+k, a] = sum_f conv_w[f,c,k]*L_w[a,f]
    mcomb = np.einsum("fck,af->cka", conv_w, L_w).reshape(KC, A)
    mcomb = np.ascontiguousarray(mcomb).astype(ml_dtypes.bfloat16)

    # processed query folded into enc (host weight algebra; tiny)
    pq = query @ W_w.T + (W_b + L_b)            # [B, A]
    encq = (enc + pq[:, None, :]).astype(ml_dtypes.bfloat16)

    eye32 = np.eye(128, dtype=np.float32)
    eyeb = np.eye(128, dtype=np.float32).astype(ml_dtypes.bfloat16)
    ones = np.ones((128, 128), np.float32)
    # tri[s,t'] = 1 for t'-2 <= s <= t'  (alpha + shift1 + shift2, in-block)
    idx = np.arange(128)
    dif = idx[None, :] - idx[:, None]          # t' - s
    tri = ((dif >= 0) & (dif <= 2)).astype(np.float32)
    # cor[s,t']: cross-block corner terms from the previous 128-block
    cor = np.zeros((128, 128), np.float32)
    cor[126, 0] = 1.0
    cor[127, 0] = 1.0
    cor[127, 1] = 1.0

    mask = (np.arange(T)[None, :] < phone_len[:, None]).astype(np.float32)

    def lay(arr):  # [8,1024] -> [128, 64] with col = k*8 + b
        return np.ascontiguousarray(
            arr.reshape(LB, NK, 128).transpose(2, 1, 0).reshape(128, LB * NK))

    in_maps = []
    for c in range(NCORES):
        sl = slice(c * LB, (c + 1) * LB)
        cum_c, att_c = cum[sl], att[sl]
        ls = np.zeros((KC, LB, T), ml_dtypes.bfloat16)
        padc = np.zeros((LB, T + 2 * PAD), np.float32)
        pada = np.zeros((LB, T + 2 * PAD), np.float32)
        padc[:, PAD:PAD + T] = cum_c
        pada[:, PAD:PAD + T] = att_c
        for k in range(KW):
            ls[k, :, :] = padc[:, k:k + T]
            ls[KW + k, :, :] = pada[:, k:k + T]

        constblob = np.zeros((128, C_W), np.float32)
        constblob[:, C_TRI:C_TRI + 128] = tri
        constblob[:, C_COR:C_COR + 128] = cor
        constblob[:, C_ONES:C_ONES + 128] = ones
        constblob[:, C_EYE:C_EYE + 128] = eye32
        constblob[:, C_MASK:C_MASK + LB * NK] = lay(mask[sl])
        constblob[:, C_ALPHA:C_ALPHA + LB * NK] = lay(alpha[sl])

        in_maps.append({
            "enc": np.ascontiguousarray(encq[sl]),
            "ls": ls,
            "mcomb": mcomb,
            "vw": np.ascontiguousarray(v_w[0].astype(ml_dtypes.bfloat16)),
            "eyeb": eyeb,
            "constblob": constblob,
        })
    return in_maps


def assemble_output(results: list[dict]) -> np.ndarray:
    out = np.empty((B, 1, T), np.float32)
    for c in range(NCORES):
        oc = results[c]["out"]                      # [(k b), 128]
        oc = oc.reshape(NK, LB, 128).transpose(1, 0, 2).reshape(LB, T)
        out[c * LB:(c + 1) * LB, 0, :] = oc
    return out


_CACHED_NC = None


def kernel(**inputs) -> np.ndarray:
    global _CACHED_NC
    if _CACHED_NC is None:
        _CACHED_NC = build_program(repeats=1)
    in_maps = prep_inputs(inputs)
    res = run_bass_kernel_spmd(_CACHED_NC, in_maps, list(range(NCORES)))
    return assemble_output(res.results)


# revision 13
# speedup vs baseline: 1013.6164x; 1.0067x over previous
"""Trainium2 Bass kernel for the forward-attention LSA step (nn_LSA_43404939494068).

Contract: kernel(**inputs) takes the FULL inputs from setup_inputs() and
returns the FULL output [64, 1, 1024] float32. Internally shards batch
across 8 NeuronCores (8 batches each), runs one Bass/Tile program SPMD.

Math notes (vs reference):
  u[b,t]   = sum_a v[a] * tanh(pq[b,a] + enc[b,t,a] + ploc[b,t,a])
  ploc     = conv1d([cumulative; attention]) @ L_w.T + L_b; the conv and the
             L-projection fold into ONE matmul: ploc[t,:] = ls[:,t].T @ M,
             M[(c,k),a] = sum_f conv_w[f,c,k] * L_w[a,f] (host-precomputed
             weight algebra), ls = 62 shifted copies of the two loc rows.
  pq+L_b+W_b is computed on host (tiny: B x A) and folded into enc, which is
  shipped bf16 (halves HBM traffic; final rel-err stays ~1e-3 << 2e-2).
  The reference's division of s=sigmoid(u) by sum(s) cancels exactly in the
  final alpha normalization, so it is skipped.

Engine placement, tuned against wall-differenced hardware timings (the
CoreSim cost model misses the ~400ns weight-load+accumulate cost of a
128-row matmul, which made the enc identity-matmul the real bottleneck):
per [128t x 2x512a] PSUM pair, PE runs the bf16 folded conv+proj matmul;
enc joins either via a bf16 identity-matmul into the same PSUM bank (2 of
4 pairs) or via a DVE tensor_add (other 2 pairs) - the 50/50 split
balances measured PE vs DVE time. One ACT tanh covers each 2-bank pair
(amortizes PSUM access latency); the x*v dot is a DVE
scalar_tensor_tensor with accum writing one u column (Pool cannot run
vector ops on NCv3 - walrus ISA check). The tail (sigmoid, premultiplied
mask*(alpha-band+eps), normalize) runs in a [t',(k,b)] 64-column layout,
then one PE transpose and a single output DMA. Input DMAs are spread
across queues: enc owns SP, first-use constants go on the ACT queue,
the rest on the gpsimd SWDGE queue; the ACT tanh table is preloaded with
a dummy activation at t=0.
"""

import sys

import numpy as np

if "/opt/trn_rl_repo" not in sys.path:
    sys.path.insert(0, "/opt/trn_rl_repo")

import concourse.bass as bass
import concourse.tile as tile
from concourse import mybir
from concourse.bass_utils import run_bass_kernel_spmd

B, T, A = 64, 1024, 512
F, KW = 32, 31
PAD = (KW - 1) // 2
NCORES = 8
LB = B // NCORES          # 8 local batches per core
NK = T // 128             # 8 t-tiles of 128
KC = 62                   # conv contraction = 2 channels * 31 taps
F32 = mybir.dt.float32
F32R = mybir.dt.float32r
BF16 = mybir.dt.bfloat16

# const blob layout (fp32, [128, 640]): tri | cor | ones | eye32 | mask | alpha
C_TRI, C_COR, C_ONES, C_EYE = 0, 128, 256, 384
C_MASK, C_ALPHA = 512, 576
C_W = 640

_MAX_WAITS = 1


def _split_sync_waits(nc):
    """walrus in this toolchain accepts at most one sync-wait per
    instruction; hoist excess waits onto NoOps inserted just before."""
    for fn in nc.m.functions:
        for blk in fn.blocks:
            new_list = []
            for inst in blk.instructions:
                si = inst.sync_info
                if si is not None and si.on_wait and len(si.on_wait) > _MAX_WAITS:
                    waits = list(si.on_wait)
                    extra, keep = waits[:-_MAX_WAITS], waits[-_MAX_WAITS:]
                    for i in range(0, len(extra), _MAX_WAITS):
                        nop = mybir.InstNoOp(
                            name=nc.get_next_instruction_name(),
                            sync_info=mybir.SyncInfo(
                                on_wait=extra[i:i + _MAX_WAITS], on_update=[]
                            ),
                            bass_nofuse=True,
                            engine=inst.engine,
                        )
                        nc.register_instruction(nop)
                        new_list.append(nop)
                    inst.sync_info = mybir.SyncInfo(
                        on_wait=keep, on_update=list(si.on_update)
                    )
                new_list.append(inst)
            blk.instructions[:] = new_list


def build_program(repeats: int = 1, hw_loop: bool = False,
                  stage: str = "full") -> bass.Bass:
    nc = bass.Bass()

    enc_d = nc.declare_dram_parameter("enc", [LB, T, A], BF16, isOutput=False)
    ls_d = nc.declare_dram_parameter("ls", [KC, LB, T], BF16, isOutput=False)
    mcomb_d = nc.declare_dram_parameter("mcomb", [KC, A], BF16, isOutput=False)
    vw_d = nc.declare_dram_parameter("vw", [A], BF16, isOutput=False)
    eyeb_d = nc.declare_dram_parameter("eyeb", [128, 128], BF16, isOutput=False)
    const_d = nc.declare_dram_parameter("constblob", [128, C_W], F32, isOutput=False)
    out_d = nc.declare_dram_parameter("out", [LB * NK, 128], F32, isOutput=True)

    TANH = mybir.ActivationFunctionType.Tanh
    SIG = mybir.ActivationFunctionType.Sigmoid
    IDENT = mybir.ActivationFunctionType.Identity
    MULT = mybir.AluOpType.mult
    ADD = mybir.AluOpType.add

    with tile.TileContext(nc) as tc:
        with (
            tc.tile_pool(name="const", bufs=1) as cpool,
            tc.tile_pool(name="encp", bufs=3) as encp,
            tc.tile_pool(name="xp", bufs=3) as xp,
            tc.tile_pool(name="xvp", bufs=2) as xvp,
            tc.tile_pool(name="tailp", bufs=2) as tailp,
            tc.tile_pool(name="zps", bufs=3, space="PSUM") as zps,
            tc.tile_pool(name="sps", bufs=2, space="PSUM") as sps,
        ):
            # ---- constants into SBUF (gpsimd/SWDGE queue; enc owns the SP
            # queue). Order = first-use order: the b0 matmuls need mcomb+eyeb
            # +ls[0] immediately; the const blob is tail-only.
            mcomb_sb = cpool.tile([KC, A], BF16, tag="mcomb")
            nc.scalar.dma_start(out=mcomb_sb[:], in_=mcomb_d[:])
            eyeb_sb = cpool.tile([128, 128], BF16, tag="eyeb")
            nc.scalar.dma_start(out=eyeb_sb[:], in_=eyeb_d[:])

            ls_sb = cpool.tile([KC, LB, T], BF16, tag="ls")
            nc.scalar.dma_start(out=ls_sb[:, 0, :], in_=ls_d[:, 0, :])

            # v broadcast to all 128 partitions (partition-step-0 DMA)
            v_sb = cpool.tile([128, A], BF16, tag="vbc")
            va = vw_d[:]
            v_bcast = bass.AP(tensor=va.tensor, offset=va.offset,
                              ap=[[0, 128]] + [list(p) for p in va.ap])
            nc.scalar.dma_start(out=v_sb[:], in_=v_bcast)

            for b in range(1, LB):
                nc.gpsimd.dma_start(out=ls_sb[:, b, :], in_=ls_d[:, b, :])
            const_sb = cpool.tile([128, C_W], F32, tag="const")
            nc.gpsimd.dma_start(out=const_sb[:], in_=const_d[:])

            u_sb = cpool.tile([128, LB * NK], F32, tag="u")
            eps_sb = cpool.tile([128, 1], F32, tag="eps")
            nc.vector.memset(eps_sb[:], 1e-7)
            warm_sb = cpool.tile([128, 1], F32, tag="warm")
            nc.scalar.activation(out=warm_sb[:], in_=eps_sb[:], func=TANH)

            tri = const_sb[:, C_TRI:C_TRI + 128]
            cor = const_sb[:, C_COR:C_COR + 128]
            ones = const_sb[:, C_ONES:C_ONES + 128]
            eye32 = const_sb[:, C_EYE:C_EYE + 128]
            mask = const_sb[:, C_MASK:C_MASK + LB * NK]
            alpha = const_sb[:, C_ALPHA:C_ALPHA + LB * NK]

            def body():
                # ---- main loop: z = ploc+(pq+enc) ; x = tanh(z) ; u = x.v ----
                for b in range(LB):
                    enc_sb = encp.tile([128, NK, A], BF16, tag="enc")
                    src_enc = enc_d[b].rearrange("(k p) a -> p k a", p=128)
                    if b == 0:
                        nc.sync.dma_start(out=enc_sb[:, 0:2, :],
                                          in_=src_enc[:, 0:2, :])
                        nc.sync.dma_start(out=enc_sb[:, 2:, :],
                                          in_=src_enc[:, 2:, :])
                    else:
                        nc.sync.dma_start(out=enc_sb[:], in_=src_enc)
                    if stage == "dma":
                        continue
                    for kp in range(NK // 2):
                        # two k-tiles share one 2-bank PSUM tile so a single
                        # tanh covers both (amortizes ACT access latency)
                        # dve_add pairs: enc joins via a DVE add instead of
                        # the PE identity-matmul (balances PE vs DVE load)
                        dve_add = ((stage in ("full", "fullh") and kp % 2 == 1)
                                   or (stage == "d1" and kp == 3)
                                   or (stage == "d3" and kp >= 1))
                        z_ps = zps.tile([128, 2, A], F32, tag="z")
                        for j in range(2):
                            k = 2 * kp + j
                            nc.tensor.matmul(
                                z_ps[:, j, :],
                                ls_sb[:, b, k * 128:(k + 1) * 128],
                                mcomb_sb[:],
                                start=True, stop=(dve_add or stage == "noeye"))
                            if not (dve_add or stage == "noeye"):
                                nc.tensor.matmul(z_ps[:, j, :], eyeb_sb[:],
                                                 enc_sb[:, k, :],
                                                 start=False, stop=True)
                        if stage == "mm":
                            continue
                        x_sb = xp.tile([128, 2, A], BF16, tag="x")
                        if dve_add:
                            xin_sb = xp.tile([128, 2, A], F32, tag="xin")
                            nc.vector.tensor_add(
                                xin_sb[:], z_ps[:],
                                enc_sb[:, 2 * kp:2 * kp + 2, :])
                            nc.scalar.activation(out=x_sb[:], in_=xin_sb[:],
                                                 func=TANH)
                        else:
                            nc.scalar.activation(out=x_sb[:], in_=z_ps[:],
                                                 func=TANH)
                        if stage == "act":
                            continue
                        for j in range(2):
                            k = 2 * kp + j
                            xv_sb = xvp.tile([128, A], BF16, tag="xv")
                            col = k * LB + b
                            nc.vector.scalar_tensor_tensor(
                                out=xv_sb[:], in0=x_sb[:, j, :], scalar=1.0,
                                in1=v_sb[:], op0=MULT, op1=MULT,
                                accum_out=u_sb[:, col:col + 1])

                if stage != "full":
                    dum_sb = tailp.tile([LB * NK, 128], F32, tag="otsb")
                    nc.vector.tensor_copy(out=dum_sb[:],
                                          in_=const_sb[0:LB * NK, 0:128])
                    nc.sync.dma_start(out=out_d[:], in_=dum_sb[:])
                    return
                # ---- tail in [t', (k,b)] layout ----
                # wm = (alpha + shift1(alpha) + shift2(alpha) + eps) * mask
                # depends only on constants: runs under the main loop, off
                # the post-u critical path (band shifts via tri/cor matmuls)
                w_ps = sps.tile([128, LB * NK], F32, tag="tailps")
                nc.tensor.matmul(w_ps[:], tri, alpha, start=True, stop=False)
                nc.tensor.matmul(w_ps[:, LB:], cor, alpha[:, :-LB],
                                 start=False, stop=True, skip_group_check=True)
                wm_sb = tailp.tile([128, LB * NK], F32, tag="wm")
                nc.vector.scalar_tensor_tensor(
                    out=wm_sb[:], in0=w_ps[:], scalar=1e-7, in1=mask,
                    op0=ADD, op1=MULT)

                s_sb = tailp.tile([128, LB * NK], F32, tag="s")
                nc.scalar.activation(out=s_sb[:], in_=u_sb[:], func=SIG)
                na_sb = tailp.tile([128, LB * NK], F32, tag="na")
                nc.vector.tensor_mul(na_sb[:], s_sb[:], wm_sb[:])

                # per-batch normalizer: colsum then sum over the k-groups
                cs_ps = sps.tile([1, LB * NK], F32, tag="tailps")
                nc.tensor.matmul(cs_ps[:], ones[:, 0:1], na_sb[:],
                                 start=True, stop=True)
                z_sb = tailp.tile([1, LB], F32, tag="zsum")
                nc.vector.tensor_reduce(
                    out=z_sb[:], in_=cs_ps.rearrange("p (k b) -> p b k", b=LB),
                    axis=mybir.AxisListType.X, op=ADD)
                rz_sb = tailp.tile([1, LB], F32, tag="rz")
                nc.vector.reciprocal(out=rz_sb[:], in_=z_sb[:])
                rz64_sb = tailp.tile([1, LB * NK], F32, tag="rz64")
                rza = rz_sb[:]
                rz_b = bass.AP(tensor=rza.tensor, offset=rza.offset,
                               ap=[list(rza.ap[0]), [0, NK], list(rza.ap[1])])
                nc.vector.tensor_copy(
                    out=rz64_sb.rearrange("p (k b) -> p k b", b=LB), in_=rz_b)
                rb_ps = sps.tile([128, LB * NK], F32, tag="tailps")
                nc.tensor.matmul(rb_ps[:], ones[0:1, :], rz64_sb[:],
                                 start=True, stop=True)
                nan_sb = tailp.tile([128, LB * NK], F32, tag="nan")
                nc.vector.tensor_mul(nan_sb[:], na_sb[:], rb_ps[:])

                # transpose to [(k b), t'] and store
                ot_ps = sps.tile([LB * NK, 128], F32, tag="tailps")
                nc.tensor.transpose(ot_ps[:], nan_sb[:], eye32)
                ot_sb = tailp.tile([LB * NK, 128], F32, tag="otsb")
                nc.vector.tensor_copy(out=ot_sb[:], in_=ot_ps[:])
                nc.sync.dma_start(out=out_d[:], in_=ot_sb[:])

            if hw_loop and repeats > 1:
                with tc.For_i(0, repeats, 1):
                    body()
            else:
                for _rep in range(repeats):
                    body()

    _split_sync_waits(nc)
    return nc


def prep_inputs(inputs: dict) -> list[dict]:
    """Full inputs -> per-core in_maps (host layout prep only)."""
    import ml_dtypes

    enc = np.asarray(inputs["encoder_seq_proj"], np.float32)
    query = np.asarray(inputs["query"], np.float32)
    cum = np.asarray(inputs["cumulative"], np.float32)
    att = np.asarray(inputs["attention"], np.float32)
    alpha = np.asarray(inputs["alpha"], np.float32)
    conv_w = np.asarray(inputs["conv_w"], np.float32)
    L_w = np.asarray(inputs["L_w"], np.float32)
    L_b = np.asarray(inputs["L_b"], np.float32)
    W_w = np.asarray(inputs["W_w"], np.float32)
    W_b = np.asarray(inputs["W_b"], np.float32)
    v_w = np.asarray(inputs["v_w"], np.float32)
    phone_len = np.asarray(inputs["phone_len"], np.int64)

    # folded conv+projection weight: M[c*31+k, a] = sum_f conv_w[f,c,k]*L_w[a,f]
    mcomb = np.einsum("fck,af->cka", conv_w, L_w).reshape(KC, A)
    mcomb = np.ascontiguousarray(mcomb).astype(ml_dtypes.bfloat16)

    # processed query folded into enc (host weight algebra; tiny)
    pq = query @ W_w.T + (W_b + L_b)            # [B, A]
    encq = (enc + pq[:, None, :]).astype(ml_dtypes.bfloat16)

    eye32 = np.eye(128, dtype=np.float32)
    eyeb = np.eye(128, dtype=np.float32).astype(ml_dtypes.bfloat16)
    ones = np.ones((128, 128), np.float32)
    # tri[s,t'] = 1 for t'-2 <= s <= t'  (alpha + shift1 + shift2, in-block)
    idx = np.arange(128)
    dif = idx[None, :] - idx[:, None]          # t' - s
    tri = ((dif >= 0) & (dif <= 2)).astype(np.float32)
    # cor[s,t']: cross-block corner terms from the previous 128-block
    cor = np.zeros((128, 128), np.float32)
    cor[126, 0] = 1.0
    cor[127, 0] = 1.0
    cor[127, 1] = 1.0

    mask = (np.arange(T)[None, :] < phone_len[:, None]).astype(np.float32)

    def lay(arr):  # [8,1024] -> [128, 64] with col = k*8 + b
        return np.ascontiguousarray(
            arr.reshape(LB, NK, 128).transpose(2, 1, 0).reshape(128, LB * NK))

    in_maps = []
    for c in range(NCORES):
        sl = slice(c * LB, (c + 1) * LB)
        cum_c, att_c = cum[sl], att[sl]
        ls = np.zeros((KC, LB, T), ml_dtypes.bfloat16)
        padc = np.zeros((LB, T + 2 * PAD), np.float32)
        pada = np.zeros((LB, T + 2 * PAD), np.float32)
        padc[:, PAD:PAD + T] = cum_c
        pada[:, PAD:PAD + T] = att_c
        for k in range(KW):
            ls[k, :, :] = padc[:, k:k + T]
            ls[KW + k, :, :] = pada[:, k:k + T]

        constblob = np.zeros((128, C_W), np.float32)
        constblob[:, C_TRI:C_TRI + 128] = tri
        constblob[:, C_COR:C_COR + 128] = cor
        constblob[:, C_ONES:C_ONES + 128] = ones
        constblob[:, C_EYE:C_EYE + 128] = eye32
        constblob[:, C_MASK:C_MASK + LB * NK] = lay(mask[sl])
        constblob[:, C_ALPHA:C_ALPHA + LB * NK] = lay(alpha[sl])

        in_maps.append({
            "enc": np.ascontiguousarray(encq[sl]),
            "ls": ls,
            "mcomb": mcomb,
            "vw": np.ascontiguousarray(v_w[0].astype(ml_dtypes.bfloat16)),
            "eyeb": eyeb,
            "constblob": constblob,
        })
    return in_maps


def assemble_output(results: list[dict]) -> np.ndarray:
    out = np.empty((B, 1, T), np.float32)
    for c in range(NCORES):
        oc = results[c]["out"]                      # [(k b), 128]
        oc = oc.reshape(NK, LB, 128).transpose(1, 0, 2).reshape(LB, T)
        out[c * LB:(c + 1) * LB, 0, :] = oc
    return out


_CACHED_NC = None


def kernel(**inputs) -> np.ndarray:
    global _CACHED_NC
    if _CACHED_NC is None:
        _CACHED_NC = build_program(repeats=1)
    in_maps = prep_inputs(inputs)
    res = run_bass_kernel_spmd(_CACHED_NC, in_maps, list(range(NCORES)))
    return assemble_output(res.results)


# revision 16
# speedup vs baseline: 1152.0247x; 1.1365x over previous
"""Trainium2 Bass kernel for the forward-attention LSA step (nn_LSA_43404939494068).

Contract: kernel(**inputs) takes the FULL inputs from setup_inputs() and
returns the FULL output [64, 1, 1024] float32. Internally shards batch
across 8 NeuronCores (8 batches each), runs one Bass/Tile program SPMD.

Math notes (vs reference):
  u[b,t]   = sum_a v[a] * tanh(pq[b,a] + enc[b,t,a] + ploc[b,t,a])
  ploc     = conv1d([cumulative; attention]) @ L_w.T + L_b; the conv and the
             L-projection fold into ONE matmul: ploc[t,:] = ls[:,t].T @ M,
             M[(c,k),a] = sum_f conv_w[f,c,k] * L_w[a,f] (host-precomputed
             weight algebra), ls = 62 shifted copies of the two loc rows.
  pq+L_b+W_b is computed on host (tiny: B x A) and folded into enc, which is
  shipped bf16 (halves HBM traffic; final rel-err stays ~1e-3 << 2e-2).
  The reference's division of s=sigmoid(u) by sum(s) cancels exactly in the
  final alpha normalization, so it is skipped.

Engine placement, tuned against wall-differenced hardware timings (the
CoreSim cost model misses the ~400ns weight-load+accumulate cost of a
128-row matmul, which made the enc identity-matmul the real bottleneck):
per [128t x 2x512a] PSUM pair, PE runs the bf16 folded conv+proj matmul;
enc joins either via a bf16 identity-matmul into the same PSUM bank (2 of
4 pairs) or via a DVE tensor_add (other 2 pairs) - the 50/50 split
balances measured PE vs DVE time. One ACT tanh covers each 2-bank pair
(amortizes PSUM access latency); the x*v dot is a DVE
scalar_tensor_tensor with accum writing one u column (Pool cannot run
vector ops on NCv3 - walrus ISA check). The tail (sigmoid, premultiplied
mask*(alpha-band+eps), normalize) runs in a [t',(k,b)] 64-column layout,
then one PE transpose and a single output DMA. Input DMAs are spread
across queues: enc owns SP, first-use constants go on the ACT queue,
the rest on the gpsimd SWDGE queue; the ACT tanh table is preloaded with
a dummy activation at t=0.
"""

import sys

import numpy as np

if "/opt/trn_rl_repo" not in sys.path:
    sys.path.insert(0, "/opt/trn_rl_repo")

import concourse.bass as bass
import concourse.tile as tile
from concourse import mybir
from concourse.bass_utils import run_bass_kernel_spmd

B, T, A = 64, 1024, 512
F, KW = 32, 31
PAD = (KW - 1) // 2
NCORES = 8
LB = B // NCORES          # 8 local batches per core
NK = T // 128             # 8 t-tiles of 128
KC = 62                   # conv contraction = 2 channels * 31 taps
F32 = mybir.dt.float32
F32R = mybir.dt.float32r
BF16 = mybir.dt.bfloat16

# const blob layout (fp32, [128, 640]): tri | cor | ones | eye32 | mask | alpha
C_TRI, C_COR, C_ONES, C_EYE = 0, 128, 256, 384
C_MASK, C_ALPHA = 512, 576
C_W = 640

_MAX_WAITS = 1


def _split_sync_waits(nc):
    """walrus in this toolchain accepts at most one sync-wait per
    instruction; hoist excess waits onto NoOps inserted just before."""
    for fn in nc.m.functions:
        for blk in fn.blocks:
            new_list = []
            for inst in blk.instructions:
                si = inst.sync_info
                if si is not None and si.on_wait and len(si.on_wait) > _MAX_WAITS:
                    waits = list(si.on_wait)
                    extra, keep = waits[:-_MAX_WAITS], waits[-_MAX_WAITS:]
                    for i in range(0, len(extra), _MAX_WAITS):
                        nop = mybir.InstNoOp(
                            name=nc.get_next_instruction_name(),
                            sync_info=mybir.SyncInfo(
                                on_wait=extra[i:i + _MAX_WAITS], on_update=[]
                            ),
                            bass_nofuse=True,
                            engine=inst.engine,
                        )
                        nc.register_instruction(nop)
                        new_list.append(nop)
                    inst.sync_info = mybir.SyncInfo(
                        on_wait=keep, on_update=list(si.on_update)
                    )
                new_list.append(inst)
            blk.instructions[:] = new_list


def build_program(repeats: int = 1, hw_loop: bool = False,
                  stage: str = "full", unroll: int = 1) -> bass.Bass:
    nc = bass.Bass()

    enc_d = nc.declare_dram_parameter("enc", [LB, T, A], BF16, isOutput=False)
    ls_d = nc.declare_dram_parameter("ls", [KC, LB, T], BF16, isOutput=False)
    mcomb_d = nc.declare_dram_parameter("mcomb", [KC, A], BF16, isOutput=False)
    vw_d = nc.declare_dram_parameter("vw", [A], BF16, isOutput=False)
    eyeb_d = nc.declare_dram_parameter("eyeb", [128, 128], BF16, isOutput=False)
    const_d = nc.declare_dram_parameter("constblob", [128, C_W], F32, isOutput=False)
    out_d = nc.declare_dram_parameter("out", [LB * NK, 128], F32, isOutput=True)

    TANH = mybir.ActivationFunctionType.Tanh
    SIG = mybir.ActivationFunctionType.Sigmoid
    IDENT = mybir.ActivationFunctionType.Identity
    MULT = mybir.AluOpType.mult
    ADD = mybir.AluOpType.add

    with tile.TileContext(nc) as tc:
        with (
            tc.tile_pool(name="const", bufs=1) as cpool,
            tc.tile_pool(name="encp", bufs=3) as encp,
            tc.tile_pool(name="xp", bufs=4) as xp,
            tc.tile_pool(name="xvp", bufs=4) as xvp,
            tc.tile_pool(name="tailp", bufs=2) as tailp,
            tc.tile_pool(name="zps", bufs=3, space="PSUM") as zps,
            tc.tile_pool(name="sps", bufs=2, space="PSUM") as sps,
        ):
            # ---- constants into SBUF (gpsimd/SWDGE queue; enc owns the SP
            # queue). Order = first-use order: the b0 matmuls need mcomb+eyeb
            # +ls[0] immediately; the const blob is tail-only.
            mcomb_sb = cpool.tile([KC, A], BF16, tag="mcomb")
            nc.scalar.dma_start(out=mcomb_sb[:], in_=mcomb_d[:])
            eyeb_sb = cpool.tile([128, 128], BF16, tag="eyeb")
            nc.scalar.dma_start(out=eyeb_sb[:], in_=eyeb_d[:])

            ls_sb = cpool.tile([KC, LB, T], BF16, tag="ls")
            nc.scalar.dma_start(out=ls_sb[:, 0, :], in_=ls_d[:, 0, :])

            # v broadcast to all 128 partitions (partition-step-0 DMA)
            v_sb = cpool.tile([128, A], BF16, tag="vbc")
            va = vw_d[:]
            v_bcast = bass.AP(tensor=va.tensor, offset=va.offset,
                              ap=[[0, 128]] + [list(p) for p in va.ap])
            nc.scalar.dma_start(out=v_sb[:], in_=v_bcast)

            for b in range(1, LB):
                nc.gpsimd.dma_start(out=ls_sb[:, b, :], in_=ls_d[:, b, :])
            const_sb = cpool.tile([128, C_W], F32, tag="const")
            nc.gpsimd.dma_start(out=const_sb[:], in_=const_d[:])

            u_sb = cpool.tile([128, LB * NK], F32, tag="u")
            eps_sb = cpool.tile([128, 1], F32, tag="eps")
            nc.vector.memset(eps_sb[:], 1e-7)
            warm_sb = cpool.tile([128, 1], F32, tag="warm")
            nc.scalar.activation(out=warm_sb[:], in_=eps_sb[:], func=TANH)

            tri = const_sb[:, C_TRI:C_TRI + 128]
            cor = const_sb[:, C_COR:C_COR + 128]
            ones = const_sb[:, C_ONES:C_ONES + 128]
            eye32 = const_sb[:, C_EYE:C_EYE + 128]
            mask = const_sb[:, C_MASK:C_MASK + LB * NK]
            alpha = const_sb[:, C_ALPHA:C_ALPHA + LB * NK]

            def body():
                # ---- main loop: z = ploc+(pq+enc) ; x = tanh(z) ; u = x.v ----
                for b in range(LB):
                    enc_sb = encp.tile([128, NK, A], BF16, tag="enc")
                    src_enc = enc_d[b].rearrange("(k p) a -> p k a", p=128)
                    if b == 0:
                        nc.sync.dma_start(out=enc_sb[:, 0:2, :],
                                          in_=src_enc[:, 0:2, :])
                        nc.sync.dma_start(out=enc_sb[:, 2:, :],
                                          in_=src_enc[:, 2:, :])
                    else:
                        nc.sync.dma_start(out=enc_sb[:], in_=src_enc)
                    if stage == "dma":
                        continue
                    for kp in range(NK // 2):
                        # two k-tiles share one 2-bank PSUM tile so a single
                        # tanh covers both (amortizes ACT access latency)
                        # dve_add pairs: enc joins via a DVE add instead of
                        # the PE identity-matmul (balances PE vs DVE load)
                        dve_add = ((stage in ("full", "d1") and kp == 3)
                                   or (stage == "fullh" and kp % 2 == 1)
                                   or (stage == "d3" and kp >= 1))
                        z_ps = zps.tile([128, 2, A], F32, tag="z")
                        for j in range(2):
                            k = 2 * kp + j
                            nc.tensor.matmul(
                                z_ps[:, j, :],
                                ls_sb[:, b, k * 128:(k + 1) * 128],
                                mcomb_sb[:],
                                start=True, stop=(dve_add or stage == "noeye"))
                            if not (dve_add or stage == "noeye"):
                                nc.tensor.matmul(z_ps[:, j, :], eyeb_sb[:],
                                                 enc_sb[:, k, :],
                                                 start=False, stop=True)
                        if stage == "mm":
                            continue
                        x_sb = xp.tile([128, 2, A], BF16, tag="x")
                        if dve_add:
                            xin_sb = xp.tile([128, 2, A], BF16, tag="xin")
                            nc.vector.tensor_add(
                                xin_sb[:], z_ps[:],
                                enc_sb[:, 2 * kp:2 * kp + 2, :])
                            nc.scalar.activation(out=x_sb[:], in_=xin_sb[:],
                                                 func=TANH)
                        else:
                            nc.scalar.activation(out=x_sb[:], in_=z_ps[:],
                                                 func=TANH)
                        if stage == "act":
                            continue
                        for j in range(2):
                            k = 2 * kp + j
                            xv_sb = xvp.tile([128, A], BF16, tag="xv")
                            col = k * LB + b
                            nc.vector.scalar_tensor_tensor(
                                out=xv_sb[:], in0=x_sb[:, j, :], scalar=1.0,
                                in1=v_sb[:], op0=MULT, op1=MULT,
                                accum_out=u_sb[:, col:col + 1])

                if stage != "full":
                    dum_sb = tailp.tile([LB * NK, 128], F32, tag="otsb")
                    nc.vector.tensor_copy(out=dum_sb[:],
                                          in_=const_sb[0:LB * NK, 0:128])
                    nc.sync.dma_start(out=out_d[:], in_=dum_sb[:])
                    return
                # ---- tail in [t', (k,b)] layout ----
                # wm = (alpha + shift1(alpha) + shift2(alpha) + eps) * mask
                # depends only on constants: runs under the main loop, off
                # the post-u critical path (band shifts via tri/cor matmuls)
                w_ps = sps.tile([128, LB * NK], F32, tag="tailps")
                nc.tensor.matmul(w_ps[:], tri, alpha, start=True, stop=False)
                nc.tensor.matmul(w_ps[:, LB:], cor, alpha[:, :-LB],
                                 start=False, stop=True, skip_group_check=True)
                wm_sb = tailp.tile([128, LB * NK], F32, tag="wm")
                nc.vector.scalar_tensor_tensor(
                    out=wm_sb[:], in0=w_ps[:], scalar=1e-7, in1=mask,
                    op0=ADD, op1=MULT)

                s_sb = tailp.tile([128, LB * NK], F32, tag="s")
                nc.scalar.activation(out=s_sb[:], in_=u_sb[:], func=SIG)
                na_sb = tailp.tile([128, LB * NK], F32, tag="na")
                nc.vector.tensor_mul(na_sb[:], s_sb[:], wm_sb[:])

                # per-batch normalizer: colsum then sum over the k-groups
                cs_ps = sps.tile([1, LB * NK], F32, tag="tailps")
                nc.tensor.matmul(cs_ps[:], ones[:, 0:1], na_sb[:],
                                 start=True, stop=True)
                z_sb = tailp.tile([1, LB], F32, tag="zsum")
                nc.vector.tensor_reduce(
                    out=z_sb[:], in_=cs_ps.rearrange("p (k b) -> p b k", b=LB),
                    axis=mybir.AxisListType.X, op=ADD)
                rz_sb = tailp.tile([1, LB], F32, tag="rz")
                nc.vector.reciprocal(out=rz_sb[:], in_=z_sb[:])
                rz64_sb = tailp.tile([1, LB * NK], F32, tag="rz64")
                rza = rz_sb[:]
                rz_b = bass.AP(tensor=rza.tensor, offset=rza.offset,
                               ap=[list(rza.ap[0]), [0, NK], list(rza.ap[1])])
                nc.vector.tensor_copy(
                    out=rz64_sb.rearrange("p (k b) -> p k b", b=LB), in_=rz_b)
                rb_ps = sps.tile([128, LB * NK], F32, tag="tailps")
                nc.tensor.matmul(rb_ps[:], ones[0:1, :], rz64_sb[:],
                                 start=True, stop=True)
                nan_sb = tailp.tile([128, LB * NK], F32, tag="nan")
                nc.vector.tensor_mul(nan_sb[:], na_sb[:], rb_ps[:])

                # transpose to [(k b), t'] and store
                ot_ps = sps.tile([LB * NK, 128], F32, tag="tailps")
                nc.tensor.transpose(ot_ps[:], nan_sb[:], eye32)
                ot_sb = tailp.tile([LB * NK, 128], F32, tag="otsb")
                nc.vector.tensor_copy(out=ot_sb[:], in_=ot_ps[:])
                nc.sync.dma_start(out=out_d[:], in_=ot_sb[:])

            if hw_loop and repeats > 1:
                assert repeats % unroll == 0
                with tc.For_i(0, repeats // unroll, 1):
                    for _u in range(unroll):
                        body()
            else:
                for _rep in range(repeats):
                    body()

    _split_sync_waits(nc)
    return nc


def prep_inputs(inputs: dict) -> list[dict]:
    """Full inputs -> per-core in_maps (host layout prep only)."""
    import ml_dtypes

    enc = np.asarray(inputs["encoder_seq_proj"], np.float32)
    query = np.asarray(inputs["query"], np.float32)
    cum = np.asarray(inputs["cumulative"], np.float32)
    att = np.asarray(inputs["attention"], np.float32)
    alpha = np.asarray(inputs["alpha"], np.float32)
    conv_w = np.asarray(inputs["conv_w"], np.float32)
    L_w = np.asarray(inputs["L_w"], np.float32)
    L_b = np.asarray(inputs["L_b"], np.float32)
    W_w = np.asarray(inputs["W_w"], np.float32)
    W_b = np.asarray(inputs["W_b"], np.float32)
    v_w = np.asarray(inputs["v_w"], np.float32)
    phone_len = np.asarray(inputs["phone_len"], np.int64)

    # folded conv+projection weight: M[c*31+k, a] = sum_f conv_w[f,c,k]*L_w[a,f]
    mcomb = np.einsum("fck,af->cka", conv_w, L_w).reshape(KC, A)
    mcomb = np.ascontiguousarray(mcomb).astype(ml_dtypes.bfloat16)

    # processed query folded into enc (host weight algebra; tiny)
    pq = query @ W_w.T + (W_b + L_b)            # [B, A]
    encq = (enc + pq[:, None, :]).astype(ml_dtypes.bfloat16)

    eye32 = np.eye(128, dtype=np.float32)
    eyeb = np.eye(128, dtype=np.float32).astype(ml_dtypes.bfloat16)
    ones = np.ones((128, 128), np.float32)
    # tri[s,t'] = 1 for t'-2 <= s <= t'  (alpha + shift1 + shift2, in-block)
    idx = np.arange(128)
    dif = idx[None, :] - idx[:, None]          # t' - s
    tri = ((dif >= 0) & (dif <= 2)).astype(np.float32)
    # cor[s,t']: cross-block corner terms from the previous 128-block
    cor = np.zeros((128, 128), np.float32)
    cor[126, 0] = 1.0
    cor[127, 0] = 1.0
    cor[127, 1] = 1.0

    mask = (np.arange(T)[None, :] < phone_len[:, None]).astype(np.float32)

    def lay(arr):  # [8,1024] -> [128, 64] with col = k*8 + b
        return np.ascontiguousarray(
            arr.reshape(LB, NK, 128).transpose(2, 1, 0).reshape(128, LB * NK))

    in_maps = []
    for c in range(NCORES):
        sl = slice(c * LB, (c + 1) * LB)
        cum_c, att_c = cum[sl], att[sl]
        ls = np.zeros((KC, LB, T), ml_dtypes.bfloat16)
        padc = np.zeros((LB, T + 2 * PAD), np.float32)
        pada = np.zeros((LB, T + 2 * PAD), np.float32)
        padc[:, PAD:PAD + T] = cum_c
        pada[:, PAD:PAD + T] = att_c
        for k in range(KW):
            ls[k, :, :] = padc[:, k:k + T]
            ls[KW + k, :, :] = pada[:, k:k + T]

        constblob = np.zeros((128, C_W), np.float32)
        constblob[:, C_TRI:C_TRI + 128] = tri
        constblob[:, C_COR:C_COR + 128] = cor
        constblob[:, C_ONES:C_ONES + 128] = ones
        constblob[:, C_EYE:C_EYE + 128] = eye32
        constblob[:, C_MASK:C_MASK + LB * NK] = lay(mask[sl])
        constblob[:, C_ALPHA:C_ALPHA + LB * NK] = lay(alpha[sl])

        in_maps.append({
            "enc": np.ascontiguousarray(encq[sl]),
            "ls": ls,
            "mcomb": mcomb,
            "vw": np.ascontiguousarray(v_w[0].astype(ml_dtypes.bfloat16)),
            "eyeb": eyeb,
            "constblob": constblob,
        })
    return in_maps


def assemble_output(results: list[dict]) -> np.ndarray:
    out = np.empty((B, 1, T), np.float32)
    for c in range(NCORES):
        oc = results[c]["out"]                      # [(k b), 128]
        oc = oc.reshape(NK, LB, 128).transpose(1, 0, 2).reshape(LB, T)
        out[c * LB:(c + 1) * LB, 0, :] = oc
    return out


_CACHED_NC = None


def kernel(**inputs) -> np.ndarray:
    global _CACHED_NC
    if _CACHED_NC is None:
        _CACHED_NC = build_program(repeats=1)
    in_maps = prep_inputs(inputs)
    res = run_bass_kernel_spmd(_CACHED_NC, in_maps, list(range(NCORES)))
    return assemble_output(res.results)


# revision 18
# speedup vs baseline: 1201.5175x; 1.0430x over previous
"""Trainium2 Bass kernel for the forward-attention LSA step (nn_LSA_43404939494068).

Contract: kernel(**inputs) takes the FULL inputs from setup_inputs() and
returns the FULL output [64, 1, 1024] float32. Internally shards batch
across 8 NeuronCores (8 batches each), runs one Bass/Tile program SPMD.

Math notes (vs reference):
  u[b,t]   = sum_a v[a] * tanh(pq[b,a] + enc[b,t,a] + ploc[b,t,a])
  ploc     = conv1d([cumulative; attention]) @ L_w.T + L_b; the conv and the
             L-projection fold into ONE matmul: ploc[t,:] = ls[:,t].T @ M,
             M[(c,k),a] = sum_f conv_w[f,c,k] * L_w[a,f] (host-precomputed
             weight algebra), ls = 62 shifted copies of the two loc rows.
  pq+L_b+W_b is computed on host (tiny: B x A) and folded into enc, which is
  shipped bf16 (halves HBM traffic; final rel-err stays ~1e-3 << 2e-2).
  The reference's division of s=sigmoid(u) by sum(s) cancels exactly in the
  final alpha normalization, so it is skipped.

Engine placement, tuned against wall-differenced hardware timings (the
CoreSim cost model misses the ~400ns weight-load+accumulate cost of a
128-row matmul, which made the enc identity-matmul the real bottleneck):
per [128t x 2x512a] PSUM pair, PE runs the bf16 folded conv+proj matmul;
enc joins either via a bf16 identity-matmul into the same PSUM bank (2 of
4 pairs) or via a DVE tensor_add (other 2 pairs) - the 50/50 split
balances measured PE vs DVE time. One ACT tanh covers each 2-bank pair
(amortizes PSUM access latency); the x*v dot is a DVE
scalar_tensor_tensor with accum writing one u column (Pool cannot run
vector ops on NCv3 - walrus ISA check). The tail (sigmoid, premultiplied
mask*(alpha-band+eps), normalize) runs in a [t',(k,b)] 64-column layout,
then one PE transpose and a single output DMA. Input DMAs are spread
across queues: enc owns SP, first-use constants go on the ACT queue,
the rest on the gpsimd SWDGE queue; the ACT tanh table is preloaded with
a dummy activation at t=0.
"""

import sys

import numpy as np

if "/opt/trn_rl_repo" not in sys.path:
    sys.path.insert(0, "/opt/trn_rl_repo")

import concourse.bass as bass
import concourse.tile as tile
from concourse import mybir
from concourse.bass_utils import run_bass_kernel_spmd

B, T, A = 64, 1024, 512
F, KW = 32, 31
PAD = (KW - 1) // 2
NCORES = 8
LB = B // NCORES          # 8 local batches per core
NK = T // 128             # 8 t-tiles of 128
KC = 62                   # conv contraction = 2 channels * 31 taps
F32 = mybir.dt.float32
F32R = mybir.dt.float32r
BF16 = mybir.dt.bfloat16

# const blob layout (fp32, [128, 640]): tri | cor | ones | eye32 | mask | alpha
C_TRI, C_COR, C_ONES, C_EYE = 0, 128, 256, 384
C_MASK, C_ALPHA = 512, 576
C_W = 640

_MAX_WAITS = 1


def _split_sync_waits(nc):
    """walrus in this toolchain accepts at most one sync-wait per
    instruction; hoist excess waits onto NoOps inserted just before."""
    for fn in nc.m.functions:
        for blk in fn.blocks:
            new_list = []
            for inst in blk.instructions:
                si = inst.sync_info
                if si is not None and si.on_wait and len(si.on_wait) > _MAX_WAITS:
                    waits = list(si.on_wait)
                    extra, keep = waits[:-_MAX_WAITS], waits[-_MAX_WAITS:]
                    for i in range(0, len(extra), _MAX_WAITS):
                        nop = mybir.InstNoOp(
                            name=nc.get_next_instruction_name(),
                            sync_info=mybir.SyncInfo(
                                on_wait=extra[i:i + _MAX_WAITS], on_update=[]
                            ),
                            bass_nofuse=True,
                            engine=inst.engine,
                        )
                        nc.register_instruction(nop)
                        new_list.append(nop)
                    inst.sync_info = mybir.SyncInfo(
                        on_wait=keep, on_update=list(si.on_update)
                    )
                new_list.append(inst)
            blk.instructions[:] = new_list


def build_program(repeats: int = 1, hw_loop: bool = False,
                  stage: str = "full", unroll: int = 1) -> bass.Bass:
    nc = bass.Bass()

    enc_d = nc.declare_dram_parameter("enc", [LB, T, A], BF16, isOutput=False)
    ls_d = nc.declare_dram_parameter("ls", [KC, LB, T], BF16, isOutput=False)
    mcomb_d = nc.declare_dram_parameter("mcomb", [KC, A], BF16, isOutput=False)
    vw_d = nc.declare_dram_parameter("vw", [A], BF16, isOutput=False)
    eyeb_d = nc.declare_dram_parameter("eyeb", [128, 128], BF16, isOutput=False)
    const_d = nc.declare_dram_parameter("constblob", [128, C_W], F32, isOutput=False)
    out_d = nc.declare_dram_parameter("out", [LB * NK, 128], F32, isOutput=True)

    TANH = mybir.ActivationFunctionType.Tanh
    SIG = mybir.ActivationFunctionType.Sigmoid
    IDENT = mybir.ActivationFunctionType.Identity
    MULT = mybir.AluOpType.mult
    ADD = mybir.AluOpType.add

    with tile.TileContext(nc) as tc:
        with (
            tc.tile_pool(name="const", bufs=1) as cpool,
            tc.tile_pool(name="encp", bufs=3) as encp,
            tc.tile_pool(name="xp", bufs=4) as xp,
            tc.tile_pool(name="tailp", bufs=2) as tailp,
            tc.tile_pool(name="zps", bufs=3, space="PSUM") as zps,
            tc.tile_pool(name="sps", bufs=2, space="PSUM") as sps,
        ):
            # ---- constants into SBUF (gpsimd/SWDGE queue; enc owns the SP
            # queue). Order = first-use order: the b0 matmuls need mcomb+eyeb
            # +ls[0] immediately; the const blob is tail-only.
            mcomb_sb = cpool.tile([KC, A], BF16, tag="mcomb")
            nc.scalar.dma_start(out=mcomb_sb[:], in_=mcomb_d[:])
            eyeb_sb = cpool.tile([128, 128], BF16, tag="eyeb")
            nc.scalar.dma_start(out=eyeb_sb[:], in_=eyeb_d[:])

            ls_sb = cpool.tile([KC, LB, T], BF16, tag="ls")
            nc.scalar.dma_start(out=ls_sb[:, 0, :], in_=ls_d[:, 0, :])

            # v broadcast to all 128 partitions (partition-step-0 DMA)
            v_sb = cpool.tile([128, A], BF16, tag="vbc")
            va = vw_d[:]
            v_bcast = bass.AP(tensor=va.tensor, offset=va.offset,
                              ap=[[0, 128]] + [list(p) for p in va.ap])
            nc.scalar.dma_start(out=v_sb[:], in_=v_bcast)

            for b in range(1, LB):
                nc.gpsimd.dma_start(out=ls_sb[:, b, :], in_=ls_d[:, b, :])
            const_sb = cpool.tile([128, C_W], F32, tag="const")
            nc.gpsimd.dma_start(out=const_sb[:], in_=const_d[:])

            u_sb = cpool.tile([128, LB * NK], F32, tag="u")
            eps_sb = cpool.tile([128, 1], F32, tag="eps")
            nc.vector.memset(eps_sb[:], 1e-7)
            warm_sb = cpool.tile([128, 1], F32, tag="warm")
            nc.scalar.activation(out=warm_sb[:], in_=eps_sb[:], func=TANH)

            tri = const_sb[:, C_TRI:C_TRI + 128]
            cor = const_sb[:, C_COR:C_COR + 128]
            ones = const_sb[:, C_ONES:C_ONES + 128]
            eye32 = const_sb[:, C_EYE:C_EYE + 128]
            mask = const_sb[:, C_MASK:C_MASK + LB * NK]
            alpha = const_sb[:, C_ALPHA:C_ALPHA + LB * NK]

            def body():
                # ---- main loop: z = ploc+(pq+enc) ; x = tanh(z) ; u = x.v ----
                for b in range(LB):
                    enc_sb = encp.tile([128, NK, A], BF16, tag="enc")
                    src_enc = enc_d[b].rearrange("(k p) a -> p k a", p=128)
                    if b == 0:
                        nc.sync.dma_start(out=enc_sb[:, 0:2, :],
                                          in_=src_enc[:, 0:2, :])
                        nc.sync.dma_start(out=enc_sb[:, 2:, :],
                                          in_=src_enc[:, 2:, :])
                    else:
                        nc.sync.dma_start(out=enc_sb[:], in_=src_enc)
                    if stage == "dma":
                        continue
                    for kp in range(NK // 2):
                        # two k-tiles share one 2-bank PSUM tile so a single
                        # tanh covers both (amortizes ACT access latency)
                        # dve_add pairs: enc joins via a DVE add instead of
                        # the PE identity-matmul (balances PE vs DVE load)
                        dve_add = ((stage in ("full", "d1") and kp == 3)
                                   or (stage == "fullh" and kp % 2 == 1)
                                   or (stage == "d3" and kp >= 1))
                        z_ps = zps.tile([128, 2, A], F32, tag="z")
                        # both mm1 first, then both identity-matmuls: the
                        # scheduler elides the second (identical) eyeb
                        # weight load when the two are adjacent
                        for j in range(2):
                            k = 2 * kp + j
                            nc.tensor.matmul(
                                z_ps[:, j, :],
                                ls_sb[:, b, k * 128:(k + 1) * 128],
                                mcomb_sb[:],
                                start=True, stop=(dve_add or stage == "noeye"),
                                skip_group_check=True)
                        if not (dve_add or stage == "noeye"):
                            for j in range(2):
                                k = 2 * kp + j
                                nc.tensor.matmul(z_ps[:, j, :], eyeb_sb[:],
                                                 enc_sb[:, k, :],
                                                 start=False, stop=True,
                                                 skip_group_check=True)
                        if stage == "mm":
                            continue
                        x_sb = xp.tile([128, 2, A], BF16, tag="x")
                        if dve_add:
                            xin_sb = xp.tile([128, 2, A], BF16, tag="xin")
                            nc.vector.tensor_add(
                                xin_sb[:], z_ps[:],
                                enc_sb[:, 2 * kp:2 * kp + 2, :])
                            nc.scalar.activation(out=x_sb[:], in_=xin_sb[:],
                                                 func=TANH)
                        else:
                            nc.scalar.activation(out=x_sb[:], in_=z_ps[:],
                                                 func=TANH)
                        if stage == "act":
                            continue
                        for j in range(2):
                            # in-place x *= v: no separate xv tile, so the
                            # stt carries no buffer-rotation wait (x is dead
                            # after this op; the x-pool WAR already orders
                            # the next tanh against it)
                            k = 2 * kp + j
                            col = k * LB + b
                            nc.vector.scalar_tensor_tensor(
                                out=x_sb[:, j, :], in0=x_sb[:, j, :],
                                scalar=1.0, in1=v_sb[:], op0=MULT, op1=MULT,
                                accum_out=u_sb[:, col:col + 1])

                if stage != "full":
                    dum_sb = tailp.tile([LB * NK, 128], F32, tag="otsb")
                    nc.vector.tensor_copy(out=dum_sb[:],
                                          in_=const_sb[0:LB * NK, 0:128])
                    nc.sync.dma_start(out=out_d[:], in_=dum_sb[:])
                    return
                # ---- tail in [t', (k,b)] layout ----
                # wm = (alpha + shift1(alpha) + shift2(alpha) + eps) * mask
                # depends only on constants: runs under the main loop, off
                # the post-u critical path (band shifts via tri/cor matmuls)
                w_ps = sps.tile([128, LB * NK], F32, tag="tailps")
                nc.tensor.matmul(w_ps[:], tri, alpha, start=True, stop=False)
                nc.tensor.matmul(w_ps[:, LB:], cor, alpha[:, :-LB],
                                 start=False, stop=True, skip_group_check=True)
                wm_sb = tailp.tile([128, LB * NK], F32, tag="wm")
                nc.vector.scalar_tensor_tensor(
                    out=wm_sb[:], in0=w_ps[:], scalar=1e-7, in1=mask,
                    op0=ADD, op1=MULT)

                s_sb = tailp.tile([128, LB * NK], F32, tag="s")
                nc.scalar.activation(out=s_sb[:], in_=u_sb[:], func=SIG)
                na_sb = tailp.tile([128, LB * NK], F32, tag="na")
                nc.vector.tensor_mul(na_sb[:], s_sb[:], wm_sb[:])

                # per-batch normalizer: colsum then sum over the k-groups
                cs_ps = sps.tile([1, LB * NK], F32, tag="tailps")
                nc.tensor.matmul(cs_ps[:], ones[:, 0:1], na_sb[:],
                                 start=True, stop=True)
                z_sb = tailp.tile([1, LB], F32, tag="zsum")
                nc.vector.tensor_reduce(
                    out=z_sb[:], in_=cs_ps.rearrange("p (k b) -> p b k", b=LB),
                    axis=mybir.AxisListType.X, op=ADD)
                rz_sb = tailp.tile([1, LB], F32, tag="rz")
                nc.vector.reciprocal(out=rz_sb[:], in_=z_sb[:])
                rz64_sb = tailp.tile([1, LB * NK], F32, tag="rz64")
                rza = rz_sb[:]
                rz_b = bass.AP(tensor=rza.tensor, offset=rza.offset,
                               ap=[list(rza.ap[0]), [0, NK], list(rza.ap[1])])
                nc.vector.tensor_copy(
                    out=rz64_sb.rearrange("p (k b) -> p k b", b=LB), in_=rz_b)
                rb_ps = sps.tile([128, LB * NK], F32, tag="tailps")
                nc.tensor.matmul(rb_ps[:], ones[0:1, :], rz64_sb[:],
                                 start=True, stop=True)
                nan_sb = tailp.tile([128, LB * NK], F32, tag="nan")
                nc.vector.tensor_mul(nan_sb[:], na_sb[:], rb_ps[:])

                # transpose to [(k b), t'] and store
                ot_ps = sps.tile([LB * NK, 128], F32, tag="tailps")
                nc.tensor.transpose(ot_ps[:], nan_sb[:], eye32)
                ot_sb = tailp.tile([LB * NK, 128], F32, tag="otsb")
                nc.vector.tensor_copy(out=ot_sb[:], in_=ot_ps[:])
                nc.sync.dma_start(out=out_d[:], in_=ot_sb[:])

            if hw_loop and repeats > 1:
                assert repeats % unroll == 0
                with tc.For_i(0, repeats // unroll, 1):
                    for _u in range(unroll):
                        body()
            else:
                for _rep in range(repeats):
                    body()

    _split_sync_waits(nc)
    return nc


def prep_inputs(inputs: dict) -> list[dict]:
    """Full inputs -> per-core in_maps (host layout prep only)."""
    import ml_dtypes

    enc = np.asarray(inputs["encoder_seq_proj"], np.float32)
    query = np.asarray(inputs["query"], np.float32)
    cum = np.asarray(inputs["cumulative"], np.float32)
    att = np.asarray(inputs["attention"], np.float32)
    alpha = np.asarray(inputs["alpha"], np.float32)
    conv_w = np.asarray(inputs["conv_w"], np.float32)
    L_w = np.asarray(inputs["L_w"], np.float32)
    L_b = np.asarray(inputs["L_b"], np.float32)
    W_w = np.asarray(inputs["W_w"], np.float32)
    W_b = np.asarray(inputs["W_b"], np.float32)
    v_w = np.asarray(inputs["v_w"], np.float32)
    phone_len = np.asarray(inputs["phone_len"], np.int64)

    # folded conv+projection weight: M[c*31+k, a] = sum_f conv_w[f,c,k]*L_w[a,f]
    mcomb = np.einsum("fck,af->cka", conv_w, L_w).reshape(KC, A)
    mcomb = np.ascontiguousarray(mcomb).astype(ml_dtypes.bfloat16)

    # processed query folded into enc (host weight algebra; tiny)
    pq = query @ W_w.T + (W_b + L_b)            # [B, A]
    encq = (enc + pq[:, None, :]).astype(ml_dtypes.bfloat16)

    eye32 = np.eye(128, dtype=np.float32)
    eyeb = np.eye(128, dtype=np.float32).astype(ml_dtypes.bfloat16)
    ones = np.ones((128, 128), np.float32)
    # tri[s,t'] = 1 for t'-2 <= s <= t'  (alpha + shift1 + shift2, in-block)
    idx = np.arange(128)
    dif = idx[None, :] - idx[:, None]          # t' - s
    tri = ((dif >= 0) & (dif <= 2)).astype(np.float32)
    # cor[s,t']: cross-block corner terms from the previous 128-block
    cor = np.zeros((128, 128), np.float32)
    cor[126, 0] = 1.0
    cor[127, 0] = 1.0
    cor[127, 1] = 1.0

    mask = (np.arange(T)[None, :] < phone_len[:, None]).astype(np.float32)

    def lay(arr):  # [8,1024] -> [128, 64] with col = k*8 + b
        return np.ascontiguousarray(
            arr.reshape(LB, NK, 128).transpose(2, 1, 0).reshape(128, LB * NK))

    in_maps = []
    for c in range(NCORES):
        sl = slice(c * LB, (c + 1) * LB)
        cum_c, att_c = cum[sl], att[sl]
        ls = np.zeros((KC, LB, T), ml_dtypes.bfloat16)
        padc = np.zeros((LB, T + 2 * PAD), np.float32)
        pada = np.zeros((LB, T + 2 * PAD), np.float32)
        padc[:, PAD:PAD + T] = cum_c
        pada[:, PAD:PAD + T] = att_c
        for k in range(KW):
            ls[k, :, :] = padc[:, k:k + T]
            ls[KW + k, :, :] = pada[:, k:k + T]

        constblob = np.zeros((128, C_W), np.float32)
        constblob[:, C_TRI:C_TRI + 128] = tri
        constblob[:, C_COR:C_COR + 128] = cor
        constblob[:, C_ONES:C_ONES + 128] = ones
        constblob[:, C_EYE:C_EYE + 128] = eye32
        constblob[:, C_MASK:C_MASK + LB * NK] = lay(mask[sl])
        constblob[:, C_ALPHA:C_ALPHA + LB * NK] = lay(alpha[sl])

        in_maps.append({
            "enc": np.ascontiguousarray(encq[sl]),
            "ls": ls,
            "mcomb": mcomb,
            "vw": np.ascontiguousarray(v_w[0].astype(ml_dtypes.bfloat16)),
            "eyeb": eyeb,
            "constblob": constblob,
        })
    return in_maps


def assemble_output(results: list[dict]) -> np.ndarray:
    out = np.empty((B, 1, T), np.float32)
    for c in range(NCORES):
        oc = results[c]["out"]                      # [(k b), 128]
        oc = oc.reshape(NK, LB, 128).transpose(1, 0, 2).reshape(LB, T)
        out[c * LB:(c + 1) * LB, 0, :] = oc
    return out


_CACHED_NC = None


def kernel(**inputs) -> np.ndarray:
    global _CACHED_NC
    if _CACHED_NC is None:
        _CACHED_NC = build_program(repeats=1)
    in_maps = prep_inputs(inputs)
    res = run_bass_kernel_spmd(_CACHED_NC, in_maps, list(range(NCORES)))
    return assemble_output(res.results)


# revision 22
# speedup vs baseline: 1204.9030x; 1.0028x over previous
"""Trainium2 Bass kernel for the forward-attention LSA step (nn_LSA_43404939494068).

Contract: kernel(**inputs) takes the FULL inputs from setup_inputs() and
returns the FULL output [64, 1, 1024] float32. Internally shards batch
across 8 NeuronCores (8 batches each), runs one Bass/Tile program SPMD.

Math notes (vs reference):
  u[b,t]   = sum_a v[a] * tanh(pq[b,a] + enc[b,t,a] + ploc[b,t,a])
  ploc     = conv1d([cumulative; attention]) @ L_w.T + L_b; the conv and the
             L-projection fold into ONE matmul: ploc[t,:] = ls[:,t].T @ M,
             M[(c,k),a] = sum_f conv_w[f,c,k] * L_w[a,f] (host-precomputed
             weight algebra), ls = 62 shifted copies of the two loc rows.
  pq+L_b+W_b is computed on host (tiny: B x A) and folded into enc, which is
  shipped bf16 (halves HBM traffic; final rel-err stays ~1e-3 << 2e-2).
  The reference's division of s=sigmoid(u) by sum(s) cancels exactly in the
  final alpha normalization, so it is skipped.

Engine placement, tuned against wall-differenced hardware timings (the
CoreSim cost model misses the ~400ns weight-load+accumulate cost of a
128-row matmul, which made the enc identity-matmul the real bottleneck):
per [128t x 2x512a] PSUM pair, PE runs the bf16 folded conv+proj matmul;
enc joins either via a bf16 identity-matmul into the same PSUM bank (2 of
4 pairs) or via a DVE tensor_add (other 2 pairs) - the 50/50 split
balances measured PE vs DVE time. One ACT tanh covers each 2-bank pair
(amortizes PSUM access latency); the x*v dot is a DVE
scalar_tensor_tensor with accum writing one u column (Pool cannot run
vector ops on NCv3 - walrus ISA check). The tail (sigmoid, premultiplied
mask*(alpha-band+eps), normalize) runs in a [t',(k,b)] 64-column layout,
then one PE transpose and a single output DMA. Input DMAs are spread
across queues: enc owns SP, first-use constants go on the ACT queue,
the rest on the gpsimd SWDGE queue; the ACT tanh table is preloaded with
a dummy activation at t=0.
"""

import sys

import numpy as np

if "/opt/trn_rl_repo" not in sys.path:
    sys.path.insert(0, "/opt/trn_rl_repo")

import concourse.bass as bass
import concourse.tile as tile
from concourse import mybir
from concourse.bass_utils import run_bass_kernel_spmd

B, T, A = 64, 1024, 512
F, KW = 32, 31
PAD = (KW - 1) // 2
NCORES = 8
LB = B // NCORES          # 8 local batches per core
NK = T // 128             # 8 t-tiles of 128
KC = 62                   # conv contraction = 2 channels * 31 taps
F32 = mybir.dt.float32
F32R = mybir.dt.float32r
BF16 = mybir.dt.bfloat16

# const blob layout (fp32, [128, 640]): tri | cor | ones | eye32 | mask | alpha
C_TRI, C_COR, C_ONES, C_EYE = 0, 128, 256, 384
C_MASK, C_ALPHA = 512, 576
C_W = 640

_MAX_WAITS = 1


def _split_sync_waits(nc):
    """walrus in this toolchain accepts at most one sync-wait per
    instruction; hoist excess waits onto NoOps inserted just before."""
    for fn in nc.m.functions:
        for blk in fn.blocks:
            new_list = []
            for inst in blk.instructions:
                si = inst.sync_info
                if si is not None and si.on_wait and len(si.on_wait) > _MAX_WAITS:
                    waits = list(si.on_wait)
                    extra, keep = waits[:-_MAX_WAITS], waits[-_MAX_WAITS:]
                    for i in range(0, len(extra), _MAX_WAITS):
                        nop = mybir.InstNoOp(
                            name=nc.get_next_instruction_name(),
                            sync_info=mybir.SyncInfo(
                                on_wait=extra[i:i + _MAX_WAITS], on_update=[]
                            ),
                            bass_nofuse=True,
                            engine=inst.engine,
                        )
                        nc.register_instruction(nop)
                        new_list.append(nop)
                    inst.sync_info = mybir.SyncInfo(
                        on_wait=keep, on_update=list(si.on_update)
                    )
                new_list.append(inst)
            blk.instructions[:] = new_list


def build_program(repeats: int = 1, hw_loop: bool = False,
                  stage: str = "full", unroll: int = 1) -> bass.Bass:
    nc = bass.Bass()

    enc_d = nc.declare_dram_parameter("enc", [LB, T, A], BF16, isOutput=False)
    ls_d = nc.declare_dram_parameter("ls", [KC, LB, T], BF16, isOutput=False)
    mcomb_d = nc.declare_dram_parameter("mcomb", [KC, A], BF16, isOutput=False)
    vw_d = nc.declare_dram_parameter("vw", [A], BF16, isOutput=False)
    eyeb_d = nc.declare_dram_parameter("eyeb", [128, 128], BF16, isOutput=False)
    const_d = nc.declare_dram_parameter("constblob", [128, C_W], F32, isOutput=False)
    out_d = nc.declare_dram_parameter("out", [LB * NK, 128], F32, isOutput=True)

    TANH = mybir.ActivationFunctionType.Tanh
    SIG = mybir.ActivationFunctionType.Sigmoid
    IDENT = mybir.ActivationFunctionType.Identity
    MULT = mybir.AluOpType.mult
    ADD = mybir.AluOpType.add

    with tile.TileContext(nc) as tc:
        with (
            tc.tile_pool(name="const", bufs=1) as cpool,
            tc.tile_pool(name="encp", bufs=4) as encp,
            tc.tile_pool(name="xp", bufs=4) as xp,
            tc.tile_pool(name="tailp", bufs=2) as tailp,
            tc.tile_pool(name="zps", bufs=3, space="PSUM") as zps,
            tc.tile_pool(name="sps", bufs=2, space="PSUM") as sps,
        ):
            # ---- constants into SBUF (gpsimd/SWDGE queue; enc owns the SP
            # queue). Order = first-use order: the b0 matmuls need mcomb+eyeb
            # +ls[0] immediately; the const blob is tail-only.
            mcomb_sb = cpool.tile([KC, A], BF16, tag="mcomb")
            nc.scalar.dma_start(out=mcomb_sb[:], in_=mcomb_d[:])
            eyeb_sb = cpool.tile([128, 128], BF16, tag="eyeb")
            nc.scalar.dma_start(out=eyeb_sb[:], in_=eyeb_d[:])

            ls_sb = cpool.tile([KC, LB, T], BF16, tag="ls")
            nc.scalar.dma_start(out=ls_sb[:, 0, :], in_=ls_d[:, 0, :])

            # v broadcast to all 128 partitions (partition-step-0 DMA)
            v_sb = cpool.tile([128, A], BF16, tag="vbc")
            va = vw_d[:]
            v_bcast = bass.AP(tensor=va.tensor, offset=va.offset,
                              ap=[[0, 128]] + [list(p) for p in va.ap])
            nc.scalar.dma_start(out=v_sb[:], in_=v_bcast)

            for b in range(1, LB):
                nc.gpsimd.dma_start(out=ls_sb[:, b, :], in_=ls_d[:, b, :])
            const_sb = cpool.tile([128, C_W], F32, tag="const")
            nc.gpsimd.dma_start(out=const_sb[:], in_=const_d[:])

            u_sb = cpool.tile([128, LB * NK], F32, tag="u")
            eps_sb = cpool.tile([128, 1], F32, tag="eps")
            nc.vector.memset(eps_sb[:], 1e-7)
            warm_sb = cpool.tile([128, 1], F32, tag="warm")
            nc.scalar.activation(out=warm_sb[:], in_=eps_sb[:], func=TANH)

            tri = const_sb[:, C_TRI:C_TRI + 128]
            cor = const_sb[:, C_COR:C_COR + 128]
            ones = const_sb[:, C_ONES:C_ONES + 128]
            eye32 = const_sb[:, C_EYE:C_EYE + 128]
            mask = const_sb[:, C_MASK:C_MASK + LB * NK]
            alpha = const_sb[:, C_ALPHA:C_ALPHA + LB * NK]

            def body():
                # ---- main loop: z = ploc+(pq+enc) ; x = tanh(z) ; u = x.v ----
                for b in range(LB):
                    enc_sb = encp.tile([128, NK, A], BF16, tag="enc")
                    src_enc = enc_d[b].rearrange("(k p) a -> p k a", p=128)
                    if b == 0:
                        nc.sync.dma_start(out=enc_sb[:, 0:2, :],
                                          in_=src_enc[:, 0:2, :])
                        nc.sync.dma_start(out=enc_sb[:, 2:, :],
                                          in_=src_enc[:, 2:, :])
                    else:
                        nc.sync.dma_start(out=enc_sb[:], in_=src_enc)
                    if stage == "dma":
                        continue
                    for kp in range(NK // 2):
                        # two k-tiles share one 2-bank PSUM tile so a single
                        # tanh covers both (amortizes ACT access latency)
                        # dve_add pairs: enc joins via a DVE add instead of
                        # the PE identity-matmul (balances PE vs DVE load)
                        dve_add = ((stage in ("full", "d1") and kp == 3)
                                   or (stage == "fullh" and kp % 2 == 1)
                                   or (stage == "d3" and kp >= 1))
                        z_ps = zps.tile([128, 2, A], F32, tag="z")
                        # both mm1 first, then both identity-matmuls: the
                        # scheduler elides the second (identical) eyeb
                        # weight load when the two are adjacent
                        for j in range(2):
                            k = 2 * kp + j
                            nc.tensor.matmul(
                                z_ps[:, j, :],
                                ls_sb[:, b, k * 128:(k + 1) * 128],
                                mcomb_sb[:],
                                start=True, stop=(dve_add or stage == "noeye"),
                                skip_group_check=True)
                        if not (dve_add or stage == "noeye"):
                            for j in range(2):
                                k = 2 * kp + j
                                nc.tensor.matmul(z_ps[:, j, :], eyeb_sb[:],
                                                 enc_sb[:, k, :],
                                                 start=False, stop=True,
                                                 skip_group_check=True)
                        if stage == "mm":
                            continue
                        x_sb = xp.tile([128, 2, A], BF16, tag="x")
                        if dve_add:
                            xin_sb = xp.tile([128, 2, A], BF16, tag="xin")
                            nc.vector.tensor_add(
                                xin_sb[:], z_ps[:],
                                enc_sb[:, 2 * kp:2 * kp + 2, :])
                            nc.scalar.activation(out=x_sb[:], in_=xin_sb[:],
                                                 func=TANH)
                        else:
                            nc.scalar.activation(out=x_sb[:], in_=z_ps[:],
                                                 func=TANH)
                        if stage == "act":
                            continue
                        for j in range(2):
                            # in-place x *= v: no separate xv tile, so the
                            # stt carries no buffer-rotation wait (x is dead
                            # after this op; the x-pool WAR already orders
                            # the next tanh against it)
                            k = 2 * kp + j
                            col = k * LB + b
                            nc.vector.scalar_tensor_tensor(
                                out=x_sb[:, j, :], in0=x_sb[:, j, :],
                                scalar=1.0, in1=v_sb[:], op0=MULT, op1=MULT,
                                accum_out=u_sb[:, col:col + 1])

                if stage != "full":
                    dum_sb = tailp.tile([LB * NK, 128], F32, tag="otsb")
                    nc.vector.tensor_copy(out=dum_sb[:],
                                          in_=const_sb[0:LB * NK, 0:128])
                    nc.sync.dma_start(out=out_d[:], in_=dum_sb[:])
                    return
                # ---- tail in [t', (k,b)] layout ----
                # wm = (alpha + shift1(alpha) + shift2(alpha) + eps) * mask
                # depends only on constants: runs under the main loop, off
                # the post-u critical path (band shifts via tri/cor matmuls)
                w_ps = sps.tile([128, LB * NK], F32, tag="tailps")
                nc.tensor.matmul(w_ps[:], tri, alpha, start=True, stop=False)
                nc.tensor.matmul(w_ps[:, LB:], cor, alpha[:, :-LB],
                                 start=False, stop=True, skip_group_check=True)
                wm_sb = tailp.tile([128, LB * NK], F32, tag="wm")
                nc.vector.scalar_tensor_tensor(
                    out=wm_sb[:], in0=w_ps[:], scalar=1e-7, in1=mask,
                    op0=ADD, op1=MULT)

                s_sb = tailp.tile([128, LB * NK], F32, tag="s")
                nc.scalar.activation(out=s_sb[:], in_=u_sb[:], func=SIG)
                na_sb = tailp.tile([128, LB * NK], F32, tag="na")
                nc.vector.tensor_mul(na_sb[:], s_sb[:], wm_sb[:])

                # per-batch normalizer: colsum then sum over the k-groups
                cs_ps = sps.tile([1, LB * NK], F32, tag="tailps")
                nc.tensor.matmul(cs_ps[:], ones[:, 0:1], na_sb[:],
                                 start=True, stop=True)
                z_sb = tailp.tile([1, LB], F32, tag="zsum")
                nc.vector.tensor_reduce(
                    out=z_sb[:], in_=cs_ps.rearrange("p (k b) -> p b k", b=LB),
                    axis=mybir.AxisListType.X, op=ADD)
                rz_sb = tailp.tile([1, LB], F32, tag="rz")
                nc.vector.reciprocal(out=rz_sb[:], in_=z_sb[:])
                rz64_sb = tailp.tile([1, LB * NK], F32, tag="rz64")
                rza = rz_sb[:]
                rz_b = bass.AP(tensor=rza.tensor, offset=rza.offset,
                               ap=[list(rza.ap[0]), [0, NK], list(rza.ap[1])])
                nc.vector.tensor_copy(
                    out=rz64_sb.rearrange("p (k b) -> p k b", b=LB), in_=rz_b)
                rb_ps = sps.tile([128, LB * NK], F32, tag="tailps")
                nc.tensor.matmul(rb_ps[:], ones[0:1, :], rz64_sb[:],
                                 start=True, stop=True)
                nan_sb = tailp.tile([128, LB * NK], F32, tag="nan")
                nc.vector.tensor_mul(nan_sb[:], na_sb[:], rb_ps[:])

                # transpose to [(k b), t'] and store
                ot_ps = sps.tile([LB * NK, 128], F32, tag="tailps")
                nc.tensor.transpose(ot_ps[:], nan_sb[:], eye32)
                ot_sb = tailp.tile([LB * NK, 128], F32, tag="otsb")
                nc.vector.tensor_copy(out=ot_sb[:], in_=ot_ps[:])
                nc.sync.dma_start(out=out_d[:], in_=ot_sb[:])

            if hw_loop and repeats > 1:
                assert repeats % unroll == 0
                with tc.For_i(0, repeats // unroll, 1):
                    for _u in range(unroll):
                        body()
            else:
                for _rep in range(repeats):
                    body()

    _split_sync_waits(nc)
    return nc


def prep_inputs(inputs: dict) -> list[dict]:
    """Full inputs -> per-core in_maps (host layout prep only)."""
    import ml_dtypes

    enc = np.asarray(inputs["encoder_seq_proj"], np.float32)
    query = np.asarray(inputs["query"], np.float32)
    cum = np.asarray(inputs["cumulative"], np.float32)
    att = np.asarray(inputs["attention"], np.float32)
    alpha = np.asarray(inputs["alpha"], np.float32)
    conv_w = np.asarray(inputs["conv_w"], np.float32)
    L_w = np.asarray(inputs["L_w"], np.float32)
    L_b = np.asarray(inputs["L_b"], np.float32)
    W_w = np.asarray(inputs["W_w"], np.float32)
    W_b = np.asarray(inputs["W_b"], np.float32)
    v_w = np.asarray(inputs["v_w"], np.float32)
    phone_len = np.asarray(inputs["phone_len"], np.int64)

    # folded conv+projection weight: M[c*31+k, a] = sum_f conv_w[f,c,k]*L_w[a,f]
    mcomb = np.einsum("fck,af->cka", conv_w, L_w).reshape(KC, A)
    mcomb = np.ascontiguousarray(mcomb).astype(ml_dtypes.bfloat16)

    # processed query folded into enc (host weight algebra; tiny)
    pq = query @ W_w.T + (W_b + L_b)            # [B, A]
    encq = (enc + pq[:, None, :]).astype(ml_dtypes.bfloat16)

    eye32 = np.eye(128, dtype=np.float32)
    eyeb = np.eye(128, dtype=np.float32).astype(ml_dtypes.bfloat16)
    ones = np.ones((128, 128), np.float32)
    # tri[s,t'] = 1 for t'-2 <= s <= t'  (alpha + shift1 + shift2, in-block)
    idx = np.arange(128)
    dif = idx[None, :] - idx[:, None]          # t' - s
    tri = ((dif >= 0) & (dif <= 2)).astype(np.float32)
    # cor[s,t']: cross-block corner terms from the previous 128-block
    cor = np.zeros((128, 128), np.float32)
    cor[126, 0] = 1.0
    cor[127, 0] = 1.0
    cor[127, 1] = 1.0

    mask = (np.arange(T)[None, :] < phone_len[:, None]).astype(np.float32)

    def lay(arr):  # [8,1024] -> [128, 64] with col = k*8 + b
        return np.ascontiguousarray(
            arr.reshape(LB, NK, 128).transpose(2, 1, 0).reshape(128, LB * NK))

    in_maps = []
    for c in range(NCORES):
        sl = slice(c * LB, (c + 1) * LB)
        cum_c, att_c = cum[sl], att[sl]
        ls = np.zeros((KC, LB, T), ml_dtypes.bfloat16)
        padc = np.zeros((LB, T + 2 * PAD), np.float32)
        pada = np.zeros((LB, T + 2 * PAD), np.float32)
        padc[:, PAD:PAD + T] = cum_c
        pada[:, PAD:PAD + T] = att_c
        for k in range(KW):
            ls[k, :, :] = padc[:, k:k + T]
            ls[KW + k, :, :] = pada[:, k:k + T]

        constblob = np.zeros((128, C_W), np.float32)
        constblob[:, C_TRI:C_TRI + 128] = tri
        constblob[:, C_COR:C_COR + 128] = cor
        constblob[:, C_ONES:C_ONES + 128] = ones
        constblob[:, C_EYE:C_EYE + 128] = eye32
        constblob[:, C_MASK:C_MASK + LB * NK] = lay(mask[sl])
        constblob[:, C_ALPHA:C_ALPHA + LB * NK] = lay(alpha[sl])

        in_maps.append({
            "enc": np.ascontiguousarray(encq[sl]),
            "ls": ls,
            "mcomb": mcomb,
            "vw": np.ascontiguousarray(v_w[0].astype(ml_dtypes.bfloat16)),
            "eyeb": eyeb,
            "constblob": constblob,
        })
    return in_maps


def assemble_output(results: list[dict]) -> np.ndarray:
    out = np.empty((B, 1, T), np.float32)
    for c in range(NCORES):
        oc = results[c]["out"]                      # [(k b), 128]
        oc = oc.reshape(NK, LB, 128).transpose(1, 0, 2).reshape(LB, T)
        out[c * LB:(c + 1) * LB, 0, :] = oc
    return out


_CACHED_NC = None


def kernel(**inputs) -> np.ndarray:
    global _CACHED_NC
    if _CACHED_NC is None:
        _CACHED_NC = build_program(repeats=1)
    in_maps = prep_inputs(inputs)
    res = run_bass_kernel_spmd(_CACHED_NC, in_maps, list(range(NCORES)))
    return assemble_output(res.results)
